# revision 24
# baseline (speedup 1.0000x reference)
"""ARMHGNN heterogeneous-GNN message-passing kernel for 8x TRN2 NeuronCores.

Data-parallel over the node batch (250 nodes/core). Feature tables replicated;
gathers done on-device via indirect DMA; content+neighbor BiLSTMs run in a
[feature-on-partition, batch-on-free] layout with partition-packed fwd/bwd
gates (fwd in partitions 0:64, bwd 64:128); recurrent matmuls use one
block-diagonal K=128 matmul per gate; biases enter PSUM via K=1 matmuls so
activations need no per-gate bias and can batch I/F/O in one sigmoid op.
Gathered rows are cast to bf16, staged to DRAM, and transposed into
[feature, row] layout by the DMA xbar (no PE transposes on the hot path).
"""

import sys

sys.path.insert(0, "/opt/trn_rl_repo")

import numpy as np
import ml_dtypes

# ---------------- problem constants (hardcoded per contract) ----------------
D = 128          # embed dim
H = 64           # LSTM hidden per direction
BATCH = 2000
NCORES = 8
BC = BATCH // NCORES          # 250 nodes per core
A_NB, P_NB, V_NB = 10, 20, 3
A_N, P_N, V_N = 100000, 200000, 1000

RE_A, RE_P, RE_V = 5 * D, 4 * D, 6 * D          # concat row elems: 640/512/768
TS_A, TS_P, TS_V = 5, 4, 6                       # content seq lens
ROWS_A = 2816    # 250 + 2500 -> pad to 22*128
ROWS_P = 5120    # 5000 -> 40*128
ROWS_V = 768     # 750 -> 6*128
NG_A, NG_P, NG_V = ROWS_A // 128, ROWS_P // 128, ROWS_V // 128   # gather tiles

# lstm index order
L_AC, L_PC, L_VC, L_AN, L_PN, L_VN = range(6)

F32 = np.float32
BF16 = ml_dtypes.bfloat16

_CACHE = {}


def _tiles_of(lo, hi):
    """[(col_offset, n_cols), ...] <=512-wide tiles covering [lo, hi)."""
    out = []
    c = lo
    while c < hi:
        n = min(512, hi - c)
        out.append((c, n))
        c += n
    return out


def _build_program():
    import concourse.bass as bass
    import concourse.tile as tile
    from concourse import bacc, mybir
    from concourse.masks import make_identity
    import contextlib

    dt = mybir.dt
    AF = mybir.ActivationFunctionType
    OP = mybir.AluOpType

    nc = bacc.Bacc("TRN2", target_bir_lowering=False, debug=False,
                   enable_asserts=False, num_devices=NCORES)

    # ---- dram io ----
    a_tab = nc.dram_tensor("a_tab", [A_N, RE_A], dt.float32, kind="ExternalInput").ap()
    p_tab = nc.dram_tensor("p_tab", [P_N, RE_P], dt.float32, kind="ExternalInput").ap()
    v_tab = nc.dram_tensor("v_tab", [V_N, RE_V], dt.float32, kind="ExternalInput").ap()
    idx_a = nc.dram_tensor("idx_a", [128, NG_A], dt.int32, kind="ExternalInput").ap()
    idx_p = nc.dram_tensor("idx_p", [128, NG_P], dt.int32, kind="ExternalInput").ap()
    idx_v = nc.dram_tensor("idx_v", [128, NG_V], dt.int32, kind="ExternalInput").ap()
    wih_d = nc.dram_tensor("wih_d", [128, 6 * 512], dt.bfloat16, kind="ExternalInput").ap()
    whh_d = nc.dram_tensor("whh_d", [128, 6 * 512], dt.bfloat16, kind="ExternalInput").ap()
    brow_d = nc.dram_tensor("brow_d", [1, 6 * 512], dt.bfloat16, kind="ExternalInput").ap()
    bvec_d = nc.dram_tensor("bvec_d", [128, 24], dt.float32, kind="ExternalInput").ap()
    att_d = nc.dram_tensor("att_d", [128, 5], dt.float32, kind="ExternalInput").ap()
    out_d = nc.dram_tensor("out_d", [BC, D], dt.float32, kind="ExternalOutput").ap()

    with tile.TileContext(nc) as tc:
        ctx = contextlib.ExitStack()
        with ctx:
            const = ctx.enter_context(tc.tile_pool(name="const", bufs=1))
            dramp = ctx.enter_context(tc.tile_pool(name="dramp", bufs=1, space="DRAM"))
            gpool = ctx.enter_context(tc.tile_pool(name="gpool", bufs=3))
            xpool = ctx.enter_context(tc.tile_pool(name="xpool", bufs=1))
            spool = ctx.enter_context(tc.tile_pool(name="spool", bufs=2))
            hcpool = ctx.enter_context(tc.tile_pool(name="hcpool", bufs=12))
            apool = ctx.enter_context(tc.tile_pool(name="apool", bufs=1))
            ppool = ctx.enter_context(tc.tile_pool(name="ppool", bufs=2, space="PSUM"))

            # ---- constants into sbuf ----
            wih = const.tile([128, 6 * 512], dt.bfloat16, name="wih", tag="wih")
            nc.sync.dma_start(wih[:], wih_d[:])
            whh = const.tile([128, 6 * 512], dt.bfloat16, name="whh", tag="whh")
            nc.sync.dma_start(whh[:], whh_d[:])
            brow = const.tile([1, 6 * 512], dt.bfloat16, name="brow", tag="brow")
            nc.sync.dma_start(brow[:], brow_d[:])
            bvec = const.tile([128, 24], dt.float32, name="bvec", tag="bvec")
            nc.sync.dma_start(bvec[:], bvec_d[:])
            attw = const.tile([128, 5], dt.float32, name="attw", tag="attw")
            nc.sync.dma_start(attw[:], att_d[:])
            ia = const.tile([128, NG_A], dt.int32, name="ia", tag="ia")
            nc.sync.dma_start(ia[:], idx_a[:])
            ip = const.tile([128, NG_P], dt.int32, name="ip", tag="ip")
            nc.sync.dma_start(ip[:], idx_p[:])
            iv = const.tile([128, NG_V], dt.int32, name="iv", tag="iv")
            nc.sync.dma_start(iv[:], idx_v[:])
            ones_row = const.tile([1, 512], dt.bfloat16, name="ones_row", tag="ones_row")
            nc.gpsimd.memset(ones_row[:], 1.0)
            ident = const.tile([128, 128], dt.float32, name="ident", tag="ident")
            make_identity(nc, ident[:])
            wb_sc = []
            for k, tsc in enumerate([TS_A, A_NB, P_NB, V_NB]):
                t_ = const.tile([1, 128], dt.float32, name=f"wbsc{k}", tag=f"wbsc{k}")
                nc.gpsimd.memset(t_[:], 1.0 / tsc)
                wb_sc.append(t_)

            # ---- DRAM staging: one tensor per (type, wave) so transpose
            # loads only depend on their own wave's stores ----
            stage = {}

            def _pair_waves(rows):
                tiles = _tiles_of(0, rows)
                n = len(tiles)
                assert n % 2 == 0
                return [[tiles[k], tiles[n - 1 - k]] for k in range(n // 2)]

            TYPES = {
                "v": dict(tab=v_tab, idx=iv, ng=NG_V, re=RE_V, rows=ROWS_V,
                          ts=TS_V, lc=L_VC, waves=_pair_waves(ROWS_V),
                          xslot=768, xbufs=8),
                "a": dict(tab=a_tab, idx=ia, ng=NG_A, re=RE_A, rows=ROWS_A,
                          ts=TS_A, lc=L_AC, waves=_pair_waves(ROWS_A),
                          xslot=1024, xbufs=7),
                "p": dict(tab=p_tab, idx=ip, ng=NG_P, re=RE_P, rows=ROWS_P,
                          ts=TS_P, lc=L_PC, waves=_pair_waves(ROWS_P),
                          xslot=1024, xbufs=6),
            }

            # ---- phase 1: gather (cast f32->bf16 in-flight) + stage ----
            # Emitted in wave order (front/back paired tiles) so each wave's
            # staging completes as early as possible; types in consume order.
            for ty in ("v", "a", "p"):
                m = TYPES[ty]
                for w, chunks in enumerate(m["waves"]):
                    wrows = sum(N for _, N in chunks)
                    st = dramp.tile([wrows, m["re"]], dt.bfloat16,
                                    name=f"stage_{ty}{w}", tag=f"st_{ty}{w}")
                    stage[(ty, w)] = st
                    loc = 0
                    for co, N in chunks:
                        for j in range(co // 128, (co + N) // 128):
                            gt = gpool.tile([128, m["re"]], dt.bfloat16,
                                            name=f"g_{ty}{j}", tag="g",
                                            padded_shape=[128, RE_V])
                            nc.gpsimd.indirect_dma_start(
                                out=gt[:], out_offset=None, in_=m["tab"][:],
                                in_offset=bass.IndirectOffsetOnAxis(
                                    ap=m["idx"][:, j:j + 1], axis=0),
                            )
                            o = loc + j * 128 - co
                            nc.gpsimd.dma_start(out=st[o:o + 128, :], in_=gt[:])
                        loc += N

            # ---- content state/accumulators ----
            # hacc_a0: f32 c_agg slice (attention); other content sums in bf16
            hacc_a0 = apool.tile([128, 256], dt.float32, name="hacc_a0", tag="hacc_a0")
            hacc_nbp = {
                "a": apool.tile([128, ROWS_A - 250], dt.bfloat16, name="hacc_an", tag="hacc_an"),
                "p": apool.tile([128, ROWS_P], dt.bfloat16, name="hacc_pn", tag="hacc_pn"),
                "v": apool.tile([128, ROWS_V], dt.bfloat16, name="hacc_vn", tag="hacc_vn"),
            }

            def hacc_segs(ty, co, N):
                """content hacc write segments: [(ap, off, n, first_is_copy)]"""
                if ty != "a":
                    return [(hacc_nbp[ty], co, N)]
                segs = []
                if co < 250:
                    n0 = min(N, 250 - co)
                    segs.append((hacc_a0, co, n0))
                    if N > n0:
                        segs.append((hacc_nbp["a"], 0, N - n0))
                else:
                    segs.append((hacc_nbp["a"], co - 250, N))
                return segs

            # ---------------- LSTM step helper ----------------
            def lstm_step(l, t, xf, xb, h_t, c_t, segs, N, tag):
                ps = ppool.tile([128, 2048], dt.float32, name=f"ps_{tag}", tag="ifog")
                first = t == 0
                for g in range(4):
                    gs = g * 512
                    nc.tensor.matmul(ps[0:64, gs:gs + N],
                                     wih[:, l * 512 + g * 128: l * 512 + g * 128 + 64],
                                     xf, start=True, stop=False, skip_group_check=True)
                    nc.tensor.matmul(ps[64:128, gs:gs + N],
                                     wih[:, l * 512 + g * 128 + 64: l * 512 + (g + 1) * 128],
                                     xb, start=True, stop=first and g == 3,
                                     skip_group_check=True)
                for g in range(3):
                    gs = g * 512
                    nc.tensor.matmul(ps[:, gs:gs + N],
                                     brow[0:1, l * 512 + g * 128: l * 512 + (g + 1) * 128],
                                     ones_row[0:1, 0:N], start=False, stop=first,
                                     skip_group_check=True)
                # recurrent matmuls last: they depend on h(t-1), so issuing
                # them after the x/bias matmuls keeps PE fed while h settles
                if not first:
                    for g in range(4):
                        gs = g * 512
                        nc.tensor.matmul(ps[:, gs:gs + N],
                                         whh[:, l * 512 + g * 128: l * 512 + (g + 1) * 128],
                                         h_t[:, 0:N], start=False, stop=True,
                                         skip_group_check=True)
                sig = spool.tile([128, 1536], dt.bfloat16, name=f"sig_{tag}", tag="sig")
                ps_ifo = ps[:, 0:1536].rearrange("p (g n) -> p g n", g=3)[:, :, 0:N]
                sg_ifo = sig[:, 0:3 * N].rearrange("p (g n) -> p g n", g=3)
                nc.scalar.activation(sg_ifo, ps_ifo, AF.Sigmoid)
                gt_ = spool.tile([128, 512], dt.bfloat16, name=f"gt_{tag}", tag="gt")
                nc.scalar.activation(gt_[:, 0:N], ps[:, 1536:1536 + N], AF.Tanh,
                                     bias=bvec[:, l * 4 + 3: l * 4 + 4])
                sI, sF, sO = sig[:, 0:N], sig[:, N:2 * N], sig[:, 2 * N:3 * N]
                if first:
                    nc.vector.tensor_tensor(out=c_t[:, 0:N], in0=sI, in1=gt_[:, 0:N],
                                            op=OP.mult)
                else:
                    tm1 = spool.tile([128, 512], dt.float32, name=f"tm1_{tag}", tag="tm1")
                    nc.vector.tensor_tensor(out=tm1[:, 0:N], in0=sF, in1=c_t[:, 0:N],
                                            op=OP.mult)
                    tm2 = spool.tile([128, 512], dt.bfloat16, name=f"tm2_{tag}", tag="tm2")
                    nc.vector.tensor_tensor(out=tm2[:, 0:N], in0=sI, in1=gt_[:, 0:N],
                                            op=OP.mult)
                    nc.vector.tensor_tensor(out=c_t[:, 0:N], in0=tm1[:, 0:N],
                                            in1=tm2[:, 0:N], op=OP.add)
                tc_ = spool.tile([128, 512], dt.bfloat16, name=f"tc_{tag}", tag="tc")
                nc.scalar.activation(tc_[:, 0:N], c_t[:, 0:N], AF.Tanh)
                nc.vector.tensor_tensor(out=h_t[:, 0:N], in0=sO, in1=tc_[:, 0:N],
                                        op=OP.mult)
                loc = 0
                for hap, off, n in segs:
                    if first:
                        nc.vector.tensor_copy(out=hap[:, off:off + n],
                                              in_=h_t[:, loc:loc + n])
                    else:
                        nc.vector.tensor_tensor(out=hap[:, off:off + n],
                                                in0=hap[:, off:off + n],
                                                in1=h_t[:, loc:loc + n], op=OP.add)
                    loc += n

            # ---------------- content + neighbor LSTMs, interleaved ----------------
            # Tile-major content chains (all T steps per tile) emitted
            # round-robin across types, with neighbor-LSTM steps interleaved
            # as soon as their input group's columns are fully accumulated.
            # This removes the serial neighbor tail and keeps PE dense.
            NB = {
                "a": dict(ln=L_AN, tnb=A_NB, goff=250),
                "p": dict(ln=L_PN, tnb=P_NB, goff=0),
                "v": dict(ln=L_VN, tnb=V_NB, goff=0),
            }
            hacc_nb = {}
            nbst = {}
            for ty in ("v", "a", "p"):
                hacc_nb[ty] = apool.tile([128, 256], dt.float32,
                                         name=f"haccnb_{ty}", tag=f"haccnb_{ty}")
                nbst[ty] = dict(
                    hn=hcpool.tile([128, 512], dt.bfloat16, name=f"hn_{ty}", tag="h"),
                    cn=hcpool.tile([128, 512], dt.float32, name=f"cn_{ty}", tag="c"))

            def emit_nb_step(ty, t):
                mb = NB[ty]
                src = hacc_nbp[ty]
                rt = mb["tnb"] - 1 - t
                lstm_step(mb["ln"], t,
                          src[:, t * BC:(t + 1) * BC],
                          src[:, rt * BC:(rt + 1) * BC],
                          nbst[ty]["hn"], nbst[ty]["cn"],
                          [(hacc_nb[ty], 0, BC)], BC, f"n{ty}{t}")

            # per-type work lists: each wave pairs a front tile and a back
            # tile so both ends of the group sequence finish early (the
            # bidirectional neighbor LSTM reads group t AND group tnb-1-t
            # at step t). Emission order must respect dataflow: a neighbor
            # step is emitted only once every column of both its groups has
            # been written by an emitted content chain.
            def build_seq(ty):
                m = TYPES[ty]
                seq = []
                for w, chunks in enumerate(m["waves"]):
                    seq.append(("loads", w))
                    for co, N in chunks:
                        seq.append(("chain", w, co, N))
                return seq

            seqs = {ty: build_seq(ty) for ty in ("v", "a", "p")}
            xw_cur = {ty: None for ty in seqs}
            covered = {ty: np.zeros(TYPES[ty]["rows"], bool) for ty in seqs}
            nb_next = {ty: 0 for ty in seqs}

            def nb_ready(ty, t):
                mb = NB[ty]
                g0 = mb["goff"] + t * BC
                g1 = mb["goff"] + (mb["tnb"] - 1 - t) * BC
                cv = covered[ty]
                return cv[g0:g0 + BC].all() and cv[g1:g1 + BC].all()

            def emit_item(ty, item):
                m = TYPES[ty]
                if item[0] == "loads":
                    w = item[1]
                    chunks = m["waves"][w]
                    st = stage[(ty, w)]
                    xw = []
                    for t in range(m["ts"]):
                        xt_t = xpool.tile([128, m["xslot"]], dt.bfloat16,
                                          name=f"xt_{ty}_{w}_{t}", tag=f"xt_{ty}",
                                          bufs=m["xbufs"])
                        loc = 0
                        for co, N in chunks:
                            nc.sync.dma_start(
                                xt_t[:, loc:loc + N],
                                st[loc:loc + N, t * 128:(t + 1) * 128],
                                transpose=True)
                            loc += N
                        xw.append(xt_t)
                    xw_cur[ty] = (w, xw)
                else:
                    _, w, co, N = item
                    w2, xw = xw_cur[ty]
                    assert w2 == w
                    loc = 0
                    for co2, N2 in m["waves"][w]:
                        if co2 == co:
                            break
                        loc += N2
                    ht = hcpool.tile([128, 512], dt.bfloat16,
                                     name=f"h_{ty}_{co}", tag="h")
                    ct = hcpool.tile([128, 512], dt.float32,
                                     name=f"c_{ty}_{co}", tag="c")
                    for t in range(m["ts"]):
                        lstm_step(m["lc"], t,
                                  xw[t][:, loc:loc + N],
                                  xw[m["ts"] - 1 - t][:, loc:loc + N],
                                  ht, ct, hacc_segs(ty, co, N), N,
                                  f"c{ty}{t}_{co}")
                    covered[ty][co:co + N] = True
                    while (nb_next[ty] < NB[ty]["tnb"]
                           and nb_ready(ty, nb_next[ty])):
                        emit_nb_step(ty, nb_next[ty])
                        nb_next[ty] += 1

            # type-sequential emission (feed order); neighbor steps interleave
            # via emit_item's readiness checks
            for ty in ("v", "a", "p"):
                for item in seqs[ty]:
                    emit_item(ty, item)

            # ---------------- attention combine ----------------
            srcs = [hacc_a0[:, 0:BC], hacc_a0[:, 0:BC], hacc_nb["a"][:, 0:BC],
                    hacc_nb["p"][:, 0:BC], hacc_nb["v"][:, 0:BC]]
            ps_s = ppool.tile([128, 2048], dt.float32, name="ps_s", tag="ifog")
            for k in range(5):
                nc.tensor.matmul(ps_s[0:1, k * 256:k * 256 + BC], attw[:, k:k + 1],
                                 srcs[k], start=True, stop=True, skip_group_check=True)
            sb_s = const.tile([1, 5 * 256], dt.float32, name="sb_s", tag="sb_s")
            for k in range(5):
                nc.vector.tensor_copy(out=sb_s[0:1, k * 256:k * 256 + BC],
                                      in_=ps_s[0:1, k * 256:k * 256 + BC])
            lr = const.tile([1, 4 * 256], dt.float32, name="lr", tag="lr")
            nc.gpsimd.memset(lr[:], 0.0)
            for k in range(4):
                nc.vector.tensor_tensor(out=lr[0:1, k * 256:k * 256 + BC],
                                        in0=sb_s[0:1, 0:BC],
                                        in1=sb_s[0:1, (k + 1) * 256:(k + 1) * 256 + BC],
                                        op=OP.add)
            lr2 = const.tile([1, 4 * 256], dt.float32, name="lr2", tag="lr2")
            nc.vector.tensor_scalar_mul(lr2[:], lr[:], 0.01)
            nc.vector.tensor_tensor(out=lr2[:], in0=lr2[:], in1=lr[:], op=OP.max)
            ex = const.tile([1, 4 * 256], dt.float32, name="ex", tag="ex")
            nc.scalar.activation(ex[:], lr2[:], AF.Exp)
            zz = const.tile([1, 256], dt.float32, name="zz", tag="zz")
            nc.vector.tensor_tensor(out=zz[0:1, 0:BC], in0=ex[0:1, 0:BC],
                                    in1=ex[0:1, 256:256 + BC], op=OP.add)
            nc.vector.tensor_tensor(out=zz[0:1, 0:BC], in0=zz[0:1, 0:BC],
                                    in1=ex[0:1, 512:512 + BC], op=OP.add)
            nc.vector.tensor_tensor(out=zz[0:1, 0:BC], in0=zz[0:1, 0:BC],
                                    in1=ex[0:1, 768:768 + BC], op=OP.add)
            rz = const.tile([1, 256], dt.float32, name="rz", tag="rz")
            nc.vector.reciprocal(rz[0:1, 0:BC], zz[0:1, 0:BC])
            wk = const.tile([1, 4 * 256], dt.float32, name="wk", tag="wk")
            for k in range(4):
                nc.vector.tensor_tensor(out=wk[0:1, k * 256:k * 256 + BC],
                                        in0=ex[0:1, k * 256:k * 256 + BC],
                                        in1=rz[0:1, 0:BC], op=OP.mult)
            esrc = [hacc_a0[:, 0:BC], hacc_nb["a"][:, 0:BC],
                    hacc_nb["p"][:, 0:BC], hacc_nb["v"][:, 0:BC]]
            facc = const.tile([128, 256], dt.float32, name="facc", tag="facc")
            prod = const.tile([128, 256], dt.float32, name="prod", tag="prod")
            ps_w = ppool.tile([128, 2048], dt.float32, name="ps_w", tag="ifog")
            for k in range(4):
                nc.tensor.matmul(ps_w[:, k * 512:k * 512 + BC], wb_sc[k][0:1, :],
                                 wk[0:1, k * 256:k * 256 + BC], start=True, stop=True,
                                 skip_group_check=True)
            for k in range(4):
                nc.vector.tensor_tensor(out=prod[:, 0:BC],
                                        in0=ps_w[:, k * 512:k * 512 + BC],
                                        in1=esrc[k], op=OP.mult)
                if k == 0:
                    nc.vector.tensor_copy(out=facc[:, 0:BC], in_=prod[:, 0:BC])
                else:
                    nc.vector.tensor_tensor(out=facc[:, 0:BC], in0=facc[:, 0:BC],
                                            in1=prod[:, 0:BC], op=OP.add)
            ps_t = ppool.tile([128, 2048], dt.float32, name="ps_t", tag="ifog")
            ot0 = const.tile([128, 128], dt.float32, name="ot0", tag="ot0")
            nc.tensor.transpose(ps_t[:, 0:128], facc[:, 0:128], ident[:])
            nc.vector.tensor_copy(out=ot0[:], in_=ps_t[:, 0:128])
            nc.sync.dma_start(out_d[0:128, :], ot0[:])
            ot1 = const.tile([128, 128], dt.float32, name="ot1", tag="ot1")
            nc.tensor.transpose(ps_t[0:BC - 128, 512:640], facc[:, 128:BC], ident[:])
            nc.vector.tensor_copy(out=ot1[0:BC - 128, :], in_=ps_t[0:BC - 128, 512:640])
            nc.sync.dma_start(out_d[128:BC, :], ot1[0:BC - 128, :])

    nc.compile()
    return nc


# =========================== host side ===========================

def _pack_lstm(p, prescale=1.0):
    """torch-gate-order params -> (wih [128,512], whh [128,512], brow [1,512]).

    col layout per gate g in order [i, f, o, g(cell)]: fwd lhsT cols 0:64,
    bwd 64:128. whh is block-diag fwd/bwd, pre-transposed for lhsT use.
    """
    rows = {0: slice(0, 64), 1: slice(64, 128), 2: slice(192, 256), 3: slice(128, 192)}
    wih = np.zeros((128, 512), F32)
    whh = np.zeros((128, 512), F32)
    brow = np.zeros((1, 512), F32)
    for g in range(4):
        r = rows[g]
        wih[:, g * 128:g * 128 + 64] = np.asarray(p["Wih_f"], F32)[r].T * prescale
        wih[:, g * 128 + 64:(g + 1) * 128] = np.asarray(p["Wih_b"], F32)[r].T * prescale
        whh[0:64, g * 128:g * 128 + 64] = np.asarray(p["Whh_f"], F32)[r].T
        whh[64:128, g * 128 + 64:(g + 1) * 128] = np.asarray(p["Whh_b"], F32)[r].T
        brow[0, g * 128:g * 128 + 64] = np.asarray(p["b_f"], F32)[r]
        brow[0, g * 128 + 64:(g + 1) * 128] = np.asarray(p["b_b"], F32)[r]
    return wih, whh, brow


def _ids_to_tiles(ids, rows):
    full = np.zeros(rows, np.int64)
    full[:len(ids)] = np.asarray(ids, np.int64)
    return np.ascontiguousarray(full.reshape(rows // 128, 128).T.astype(np.int32))


def make_in_maps(feats, rnn_a_content, rnn_p_content, rnn_v_content,
                 rnn_a_neigh, rnn_p_neigh, rnn_v_neigh, a_neigh_att,
                 id_batch, a_neigh_ids, p_neigh_ids, v_neigh_ids):
    f = {k: np.asarray(v, F32) for k, v in feats.items()}
    a_tab = np.ascontiguousarray(np.concatenate(
        [f["a_coop"], f["a_net"], f["a_text"]], axis=1))
    p_tab = np.ascontiguousarray(np.concatenate(
        [f["p_t"], f["p_v_net"], f["p_a_net"], f["p_net"]], axis=1))
    v_tab = np.ascontiguousarray(np.concatenate(
        [f["v_net"], f["v_text"]], axis=1))

    packs = [
        _pack_lstm(rnn_a_content), _pack_lstm(rnn_p_content), _pack_lstm(rnn_v_content),
        _pack_lstm(rnn_a_neigh, 1.0 / TS_A), _pack_lstm(rnn_p_neigh, 1.0 / TS_P),
        _pack_lstm(rnn_v_neigh, 1.0 / TS_V),
    ]
    wih = np.concatenate([p[0] for p in packs], axis=1).astype(BF16)
    whh = np.concatenate([p[1] for p in packs], axis=1).astype(BF16)
    brow = np.concatenate([p[2] for p in packs], axis=1).astype(BF16)
    # per-partition bias vectors [128, 6*4]: col l*4+g = [b_fwd(64); b_bwd(64)]
    bvec = np.zeros((128, 24), F32)
    for l in range(6):
        br = packs[l][2][0]
        for g in range(4):
            bvec[0:64, l * 4 + g] = br[g * 128:g * 128 + 64]
            bvec[64:128, l * 4 + g] = br[g * 128 + 64:(g + 1) * 128]

    att = np.asarray(a_neigh_att, F32)          # [2d, 1]
    attA, attB = att[0:128, 0], att[128:256, 0]
    attw = np.stack([attA / TS_A, attB / TS_A, attB / A_NB, attB / P_NB,
                     attB / V_NB], axis=1).astype(F32)     # [128, 5]

    idb = np.asarray(id_batch, np.int64)
    anb = np.asarray(a_neigh_ids, np.int64)
    pnb = np.asarray(p_neigh_ids, np.int64)
    vnb = np.asarray(v_neigh_ids, np.int64)

    in_maps = []
    for c in range(NCORES):
        s = slice(c * BC, (c + 1) * BC)
        ids_a = np.concatenate([idb[s], anb[s].T.ravel()])   # slot-major (t-major)
        in_maps.append({
            "a_tab": a_tab, "p_tab": p_tab, "v_tab": v_tab,
            "idx_a": _ids_to_tiles(ids_a, ROWS_A),
            "idx_p": _ids_to_tiles(pnb[s].T.ravel(), ROWS_P),
            "idx_v": _ids_to_tiles(vnb[s].T.ravel(), ROWS_V),
            "wih_d": wih, "whh_d": whh, "brow_d": brow, "bvec_d": bvec.copy(),
            "att_d": attw,
        })
    return in_maps


def get_program():
    if "nc" not in _CACHE:
        _CACHE["nc"] = _build_program()
    return _CACHE["nc"]


def kernel(**inputs):
    from concourse import bass_utils

    nc = get_program()
    in_maps = make_in_maps(**inputs)
    res = bass_utils.run_bass_kernel_spmd(nc, in_maps, core_ids=list(range(NCORES)))
    out = np.concatenate([res.results[c]["out_d"] for c in range(NCORES)], axis=0)
    return np.asarray(out, F32)


# revision 27
# speedup vs baseline: 1.1842x; 1.1842x over previous
"""ARMHGNN heterogeneous-GNN message-passing kernel for 8x TRN2 NeuronCores.

Data-parallel over the node batch (250 nodes/core). Feature tables replicated;
gathers done on-device via indirect DMA; content+neighbor BiLSTMs run in a
[feature-on-partition, batch-on-free] layout with partition-packed fwd/bwd
gates (fwd in partitions 0:64, bwd 64:128); recurrent matmuls use one
block-diagonal K=128 matmul per gate; biases enter PSUM via K=1 matmuls so
activations need no per-gate bias and can batch I/F/O in one sigmoid op.
Gathered rows are cast to bf16, staged to DRAM, and transposed into
[feature, row] layout by the DMA xbar (no PE transposes on the hot path).
"""

import sys

sys.path.insert(0, "/opt/trn_rl_repo")

import numpy as np
import ml_dtypes

# ---------------- problem constants (hardcoded per contract) ----------------
D = 128          # embed dim
H = 64           # LSTM hidden per direction
BATCH = 2000
NCORES = 8
BC = BATCH // NCORES          # 250 nodes per core
A_NB, P_NB, V_NB = 10, 20, 3
A_N, P_N, V_N = 100000, 200000, 1000

RE_A, RE_P, RE_V = 5 * D, 4 * D, 6 * D          # concat row elems: 640/512/768
TS_A, TS_P, TS_V = 5, 4, 6                       # content seq lens
ROWS_A = 2816    # 250 + 2500 -> pad to 22*128
ROWS_P = 5120    # 5000 -> 40*128
ROWS_V = 768     # 750 -> 6*128
NG_A, NG_P, NG_V = ROWS_A // 128, ROWS_P // 128, ROWS_V // 128   # gather tiles

# lstm index order
L_AC, L_PC, L_VC, L_AN, L_PN, L_VN = range(6)

F32 = np.float32
BF16 = ml_dtypes.bfloat16

_CACHE = {}


def _tiles_of(lo, hi):
    """[(col_offset, n_cols), ...] <=512-wide tiles covering [lo, hi)."""
    out = []
    c = lo
    while c < hi:
        n = min(512, hi - c)
        out.append((c, n))
        c += n
    return out


def _build_program():
    import concourse.bass as bass
    import concourse.tile as tile
    from concourse import bacc, mybir
    from concourse.masks import make_identity
    import contextlib

    dt = mybir.dt
    AF = mybir.ActivationFunctionType
    OP = mybir.AluOpType

    nc = bacc.Bacc("TRN2", target_bir_lowering=False, debug=False,
                   enable_asserts=False, num_devices=NCORES)

    # ---- dram io ----
    a_tab = nc.dram_tensor("a_tab", [A_N, RE_A], dt.float32, kind="ExternalInput").ap()
    p_tab = nc.dram_tensor("p_tab", [P_N, RE_P], dt.float32, kind="ExternalInput").ap()
    v_tab = nc.dram_tensor("v_tab", [V_N, RE_V], dt.float32, kind="ExternalInput").ap()
    idx_a = nc.dram_tensor("idx_a", [128, NG_A], dt.int32, kind="ExternalInput").ap()
    idx_p = nc.dram_tensor("idx_p", [128, NG_P], dt.int32, kind="ExternalInput").ap()
    idx_v = nc.dram_tensor("idx_v", [128, NG_V], dt.int32, kind="ExternalInput").ap()
    wih_d = nc.dram_tensor("wih_d", [128, 6 * 512], dt.bfloat16, kind="ExternalInput").ap()
    whh_d = nc.dram_tensor("whh_d", [128, 6 * 512], dt.bfloat16, kind="ExternalInput").ap()
    brow_d = nc.dram_tensor("brow_d", [1, 6 * 512], dt.bfloat16, kind="ExternalInput").ap()
    bvec_d = nc.dram_tensor("bvec_d", [128, 24], dt.float32, kind="ExternalInput").ap()
    att_d = nc.dram_tensor("att_d", [128, 5], dt.float32, kind="ExternalInput").ap()
    out_d = nc.dram_tensor("out_d", [BC, D], dt.float32, kind="ExternalOutput").ap()

    with tile.TileContext(nc) as tc:
        ctx = contextlib.ExitStack()
        with ctx:
            const = ctx.enter_context(tc.tile_pool(name="const", bufs=1))
            dramp = ctx.enter_context(tc.tile_pool(name="dramp", bufs=1, space="DRAM"))
            gpool = ctx.enter_context(tc.tile_pool(name="gpool", bufs=3))
            xpool = ctx.enter_context(tc.tile_pool(name="xpool", bufs=1))
            spool = ctx.enter_context(tc.tile_pool(name="spool", bufs=2))
            hcpool = ctx.enter_context(tc.tile_pool(name="hcpool", bufs=12))
            apool = ctx.enter_context(tc.tile_pool(name="apool", bufs=1))
            ppool = ctx.enter_context(tc.tile_pool(name="ppool", bufs=2, space="PSUM"))

            # ---- constants into sbuf ----
            wih = const.tile([128, 6 * 512], dt.bfloat16, name="wih", tag="wih")
            nc.sync.dma_start(wih[:], wih_d[:])
            whh = const.tile([128, 6 * 512], dt.bfloat16, name="whh", tag="whh")
            nc.sync.dma_start(whh[:], whh_d[:])
            brow = const.tile([1, 6 * 512], dt.bfloat16, name="brow", tag="brow")
            nc.sync.dma_start(brow[:], brow_d[:])
            bvec = const.tile([128, 24], dt.float32, name="bvec", tag="bvec")
            nc.sync.dma_start(bvec[:], bvec_d[:])
            attw = const.tile([128, 5], dt.float32, name="attw", tag="attw")
            nc.sync.dma_start(attw[:], att_d[:])
            ia = const.tile([128, NG_A], dt.int32, name="ia", tag="ia")
            nc.sync.dma_start(ia[:], idx_a[:])
            ip = const.tile([128, NG_P], dt.int32, name="ip", tag="ip")
            nc.sync.dma_start(ip[:], idx_p[:])
            iv = const.tile([128, NG_V], dt.int32, name="iv", tag="iv")
            nc.sync.dma_start(iv[:], idx_v[:])
            ones_row = const.tile([1, 512], dt.bfloat16, name="ones_row", tag="ones_row")
            nc.gpsimd.memset(ones_row[:], 1.0)
            ident = const.tile([128, 128], dt.float32, name="ident", tag="ident")
            make_identity(nc, ident[:])
            wb_sc = []
            for k, tsc in enumerate([TS_A, A_NB, P_NB, V_NB]):
                t_ = const.tile([1, 128], dt.float32, name=f"wbsc{k}", tag=f"wbsc{k}")
                nc.gpsimd.memset(t_[:], 1.0 / tsc)
                wb_sc.append(t_)

            # ---- DRAM staging: one tensor per (type, wave) so transpose
            # loads only depend on their own wave's stores ----
            stage = {}

            def _pair_waves(rows, k=2):
                """waves of up to 2k tiles taken from both ends inward, so the
                bidirectional neighbor LSTM's input groups finish early."""
                tiles = _tiles_of(0, rows)
                waves = []
                lo, hi = 0, len(tiles)
                while lo < hi:
                    kk = min(k, (hi - lo + 1) // 2)
                    w = tiles[lo:lo + kk] + tiles[max(lo + kk, hi - kk):hi]
                    waves.append(w)
                    lo += kk
                    hi = max(lo, hi - kk)
                return waves

            TYPES = {
                "v": dict(tab=v_tab, idx=iv, ng=NG_V, re=RE_V, rows=ROWS_V,
                          ts=TS_V, lc=L_VC, waves=_pair_waves(ROWS_V),
                          xslot=768, xbufs=TS_V + 2),
                "a": dict(tab=a_tab, idx=ia, ng=NG_A, re=RE_A, rows=ROWS_A,
                          ts=TS_A, lc=L_AC, waves=_pair_waves(ROWS_A),
                          xslot=1792, xbufs=TS_A + 2),
                "p": dict(tab=p_tab, idx=ip, ng=NG_P, re=RE_P, rows=ROWS_P,
                          ts=TS_P, lc=L_PC, waves=_pair_waves(ROWS_P),
                          xslot=2048, xbufs=TS_P + 2),
            }

            # ---- phase 1: gather (cast f32->bf16 in-flight) + stage ----
            # Emitted in wave order (front/back paired tiles) so each wave's
            # staging completes as early as possible; types in consume order.
            for ty in ("v", "a", "p"):
                m = TYPES[ty]
                for w, chunks in enumerate(m["waves"]):
                    wrows = sum(N for _, N in chunks)
                    st = dramp.tile([wrows, m["re"]], dt.bfloat16,
                                    name=f"stage_{ty}{w}", tag=f"st_{ty}{w}")
                    stage[(ty, w)] = st
                    loc = 0
                    for co, N in chunks:
                        for j in range(co // 128, (co + N) // 128):
                            gt = gpool.tile([128, m["re"]], dt.bfloat16,
                                            name=f"g_{ty}{j}", tag="g",
                                            padded_shape=[128, RE_V])
                            nc.gpsimd.indirect_dma_start(
                                out=gt[:], out_offset=None, in_=m["tab"][:],
                                in_offset=bass.IndirectOffsetOnAxis(
                                    ap=m["idx"][:, j:j + 1], axis=0),
                            )
                            o = loc + j * 128 - co
                            nc.gpsimd.dma_start(out=st[o:o + 128, :], in_=gt[:])
                        loc += N

            # ---- content state/accumulators ----
            # hacc_a0: f32 c_agg slice (attention); other content sums in bf16
            hacc_a0 = apool.tile([128, 256], dt.float32, name="hacc_a0", tag="hacc_a0")
            hacc_nbp = {
                "a": apool.tile([128, ROWS_A - 250], dt.bfloat16, name="hacc_an", tag="hacc_an"),
                "p": apool.tile([128, ROWS_P], dt.bfloat16, name="hacc_pn", tag="hacc_pn"),
                "v": apool.tile([128, ROWS_V], dt.bfloat16, name="hacc_vn", tag="hacc_vn"),
            }

            def hacc_segs(ty, co, N):
                """content hacc write segments: [(ap, off, n, first_is_copy)]"""
                if ty != "a":
                    return [(hacc_nbp[ty], co, N)]
                segs = []
                if co < 250:
                    n0 = min(N, 250 - co)
                    segs.append((hacc_a0, co, n0))
                    if N > n0:
                        segs.append((hacc_nbp["a"], 0, N - n0))
                else:
                    segs.append((hacc_nbp["a"], co - 250, N))
                return segs

            # ---------------- LSTM step helper ----------------
            def lstm_step(l, t, xf, xb, h_t, c_t, segs, N, tag):
                ps = ppool.tile([128, 2048], dt.float32, name=f"ps_{tag}", tag="ifog")
                first = t == 0
                for g in range(4):
                    gs = g * 512
                    nc.tensor.matmul(ps[0:64, gs:gs + N],
                                     wih[:, l * 512 + g * 128: l * 512 + g * 128 + 64],
                                     xf, start=True, stop=False, skip_group_check=True)
                    nc.tensor.matmul(ps[64:128, gs:gs + N],
                                     wih[:, l * 512 + g * 128 + 64: l * 512 + (g + 1) * 128],
                                     xb, start=True, stop=first and g == 3,
                                     skip_group_check=True)
                for g in range(3):
                    gs = g * 512
                    nc.tensor.matmul(ps[:, gs:gs + N],
                                     brow[0:1, l * 512 + g * 128: l * 512 + (g + 1) * 128],
                                     ones_row[0:1, 0:N], start=False, stop=first,
                                     skip_group_check=True)
                # recurrent matmuls last: they depend on h(t-1), so issuing
                # them after the x/bias matmuls keeps PE fed while h settles
                if not first:
                    for g in range(4):
                        gs = g * 512
                        nc.tensor.matmul(ps[:, gs:gs + N],
                                         whh[:, l * 512 + g * 128: l * 512 + (g + 1) * 128],
                                         h_t[:, 0:N], start=False, stop=True,
                                         skip_group_check=True)
                sig = spool.tile([128, 1536], dt.bfloat16, name=f"sig_{tag}", tag="sig")
                ps_ifo = ps[:, 0:1536].rearrange("p (g n) -> p g n", g=3)[:, :, 0:N]
                sg_ifo = sig[:, 0:3 * N].rearrange("p (g n) -> p g n", g=3)
                nc.scalar.activation(sg_ifo, ps_ifo, AF.Sigmoid)
                gt_ = spool.tile([128, 512], dt.bfloat16, name=f"gt_{tag}", tag="gt")
                nc.scalar.activation(gt_[:, 0:N], ps[:, 1536:1536 + N], AF.Tanh,
                                     bias=bvec[:, l * 4 + 3: l * 4 + 4])
                sI, sF, sO = sig[:, 0:N], sig[:, N:2 * N], sig[:, 2 * N:3 * N]
                if first:
                    nc.vector.tensor_tensor(out=c_t[:, 0:N], in0=sI, in1=gt_[:, 0:N],
                                            op=OP.mult)
                else:
                    tm1 = spool.tile([128, 512], dt.float32, name=f"tm1_{tag}", tag="tm1")
                    nc.vector.tensor_tensor(out=tm1[:, 0:N], in0=sF, in1=c_t[:, 0:N],
                                            op=OP.mult)
                    tm2 = spool.tile([128, 512], dt.bfloat16, name=f"tm2_{tag}", tag="tm2")
                    nc.vector.tensor_tensor(out=tm2[:, 0:N], in0=sI, in1=gt_[:, 0:N],
                                            op=OP.mult)
                    nc.vector.tensor_tensor(out=c_t[:, 0:N], in0=tm1[:, 0:N],
                                            in1=tm2[:, 0:N], op=OP.add)
                tc_ = spool.tile([128, 512], dt.bfloat16, name=f"tc_{tag}", tag="tc")
                nc.scalar.activation(tc_[:, 0:N], c_t[:, 0:N], AF.Tanh)
                nc.vector.tensor_tensor(out=h_t[:, 0:N], in0=sO, in1=tc_[:, 0:N],
                                        op=OP.mult)
                loc = 0
                for hap, off, n in segs:
                    if first:
                        nc.vector.tensor_copy(out=hap[:, off:off + n],
                                              in_=h_t[:, loc:loc + n])
                    else:
                        nc.vector.tensor_tensor(out=hap[:, off:off + n],
                                                in0=hap[:, off:off + n],
                                                in1=h_t[:, loc:loc + n], op=OP.add)
                    loc += n

            # ---------------- content + neighbor LSTMs, interleaved ----------------
            # Tile-major content chains (all T steps per tile) emitted
            # round-robin across types, with neighbor-LSTM steps interleaved
            # as soon as their input group's columns are fully accumulated.
            # This removes the serial neighbor tail and keeps PE dense.
            NB = {
                "a": dict(ln=L_AN, tnb=A_NB, goff=250),
                "p": dict(ln=L_PN, tnb=P_NB, goff=0),
                "v": dict(ln=L_VN, tnb=V_NB, goff=0),
            }
            hacc_nb = {}
            nbst = {}
            for ty in ("v", "a", "p"):
                hacc_nb[ty] = apool.tile([128, 256], dt.float32,
                                         name=f"haccnb_{ty}", tag=f"haccnb_{ty}")
                nbst[ty] = dict(
                    hn=hcpool.tile([128, 512], dt.bfloat16, name=f"hn_{ty}", tag="h"),
                    cn=hcpool.tile([128, 512], dt.float32, name=f"cn_{ty}", tag="c"))

            def emit_nb_step(ty, t):
                mb = NB[ty]
                src = hacc_nbp[ty]
                rt = mb["tnb"] - 1 - t
                lstm_step(mb["ln"], t,
                          src[:, t * BC:(t + 1) * BC],
                          src[:, rt * BC:(rt + 1) * BC],
                          nbst[ty]["hn"], nbst[ty]["cn"],
                          [(hacc_nb[ty], 0, BC)], BC, f"n{ty}{t}")

            # per-type work lists: each wave pairs a front tile and a back
            # tile so both ends of the group sequence finish early (the
            # bidirectional neighbor LSTM reads group t AND group tnb-1-t
            # at step t). Emission order must respect dataflow: a neighbor
            # step is emitted only once every column of both its groups has
            # been written by an emitted content chain.
            def build_seq(ty):
                m = TYPES[ty]
                seq = []
                for w in range(len(m["waves"])):
                    seq.append(("loads", w))
                    seq.append(("wave", w))
                return seq

            seqs = {ty: build_seq(ty) for ty in ("v", "a", "p")}
            xw_cur = {ty: None for ty in seqs}
            covered = {ty: np.zeros(TYPES[ty]["rows"], bool) for ty in seqs}
            nb_next = {ty: 0 for ty in seqs}

            def nb_ready(ty, t):
                mb = NB[ty]
                g0 = mb["goff"] + t * BC
                g1 = mb["goff"] + (mb["tnb"] - 1 - t) * BC
                cv = covered[ty]
                return cv[g0:g0 + BC].all() and cv[g1:g1 + BC].all()

            def emit_item(ty, item):
                m = TYPES[ty]
                if item[0] == "loads":
                    w = item[1]
                    chunks = m["waves"][w]
                    st = stage[(ty, w)]
                    xw = []
                    for t in range(m["ts"]):
                        xt_t = xpool.tile([128, m["xslot"]], dt.bfloat16,
                                          name=f"xt_{ty}_{w}_{t}", tag=f"xt_{ty}",
                                          bufs=m["xbufs"])
                        loc = 0
                        for co, N in chunks:
                            nc.sync.dma_start(
                                xt_t[:, loc:loc + N],
                                st[loc:loc + N, t * 128:(t + 1) * 128],
                                transpose=True)
                            loc += N
                        xw.append(xt_t)
                    xw_cur[ty] = (w, xw)
                else:
                    # t-major over the whole wave: all chunks' step-t matmuls
                    # issue together, so the in-order PE queue never
                    # head-of-line-blocks on a single chain's recurrence
                    w = item[1]
                    w2, xw = xw_cur[ty]
                    assert w2 == w
                    chunks = m["waves"][w]
                    locs = []
                    loc = 0
                    for co, N in chunks:
                        locs.append(loc)
                        loc += N
                    hts = [hcpool.tile([128, 512], dt.bfloat16,
                                       name=f"h_{ty}_{co}", tag="h")
                           for co, N in chunks]
                    cts = [hcpool.tile([128, 512], dt.float32,
                                       name=f"c_{ty}_{co}", tag="c")
                           for co, N in chunks]
                    for t in range(m["ts"]):
                        for i, (co, N) in enumerate(chunks):
                            lstm_step(m["lc"], t,
                                      xw[t][:, locs[i]:locs[i] + N],
                                      xw[m["ts"] - 1 - t][:, locs[i]:locs[i] + N],
                                      hts[i], cts[i], hacc_segs(ty, co, N), N,
                                      f"c{ty}{t}_{co}")
                    for co, N in chunks:
                        covered[ty][co:co + N] = True
                    while (nb_next[ty] < NB[ty]["tnb"]
                           and nb_ready(ty, nb_next[ty])):
                        emit_nb_step(ty, nb_next[ty])
                        nb_next[ty] += 1

            # type-sequential emission (feed order); neighbor steps interleave
            # via emit_item's readiness checks
            for ty in ("v", "a", "p"):
                for item in seqs[ty]:
                    emit_item(ty, item)

            # ---------------- attention combine ----------------
            srcs = [hacc_a0[:, 0:BC], hacc_a0[:, 0:BC], hacc_nb["a"][:, 0:BC],
                    hacc_nb["p"][:, 0:BC], hacc_nb["v"][:, 0:BC]]
            ps_s = ppool.tile([128, 2048], dt.float32, name="ps_s", tag="ifog")
            for k in range(5):
                nc.tensor.matmul(ps_s[0:1, k * 256:k * 256 + BC], attw[:, k:k + 1],
                                 srcs[k], start=True, stop=True, skip_group_check=True)
            sb_s = const.tile([1, 5 * 256], dt.float32, name="sb_s", tag="sb_s")
            for k in range(5):
                nc.vector.tensor_copy(out=sb_s[0:1, k * 256:k * 256 + BC],
                                      in_=ps_s[0:1, k * 256:k * 256 + BC])
            lr = const.tile([1, 4 * 256], dt.float32, name="lr", tag="lr")
            nc.gpsimd.memset(lr[:], 0.0)
            for k in range(4):
                nc.vector.tensor_tensor(out=lr[0:1, k * 256:k * 256 + BC],
                                        in0=sb_s[0:1, 0:BC],
                                        in1=sb_s[0:1, (k + 1) * 256:(k + 1) * 256 + BC],
                                        op=OP.add)
            lr2 = const.tile([1, 4 * 256], dt.float32, name="lr2", tag="lr2")
            nc.vector.tensor_scalar_mul(lr2[:], lr[:], 0.01)
            nc.vector.tensor_tensor(out=lr2[:], in0=lr2[:], in1=lr[:], op=OP.max)
            ex = const.tile([1, 4 * 256], dt.float32, name="ex", tag="ex")
            nc.scalar.activation(ex[:], lr2[:], AF.Exp)
            zz = const.tile([1, 256], dt.float32, name="zz", tag="zz")
            nc.vector.tensor_tensor(out=zz[0:1, 0:BC], in0=ex[0:1, 0:BC],
                                    in1=ex[0:1, 256:256 + BC], op=OP.add)
            nc.vector.tensor_tensor(out=zz[0:1, 0:BC], in0=zz[0:1, 0:BC],
                                    in1=ex[0:1, 512:512 + BC], op=OP.add)
            nc.vector.tensor_tensor(out=zz[0:1, 0:BC], in0=zz[0:1, 0:BC],
                                    in1=ex[0:1, 768:768 + BC], op=OP.add)
            rz = const.tile([1, 256], dt.float32, name="rz", tag="rz")
            nc.vector.reciprocal(rz[0:1, 0:BC], zz[0:1, 0:BC])
            wk = const.tile([1, 4 * 256], dt.float32, name="wk", tag="wk")
            for k in range(4):
                nc.vector.tensor_tensor(out=wk[0:1, k * 256:k * 256 + BC],
                                        in0=ex[0:1, k * 256:k * 256 + BC],
                                        in1=rz[0:1, 0:BC], op=OP.mult)
            esrc = [hacc_a0[:, 0:BC], hacc_nb["a"][:, 0:BC],
                    hacc_nb["p"][:, 0:BC], hacc_nb["v"][:, 0:BC]]
            facc = const.tile([128, 256], dt.float32, name="facc", tag="facc")
            prod = const.tile([128, 256], dt.float32, name="prod", tag="prod")
            ps_w = ppool.tile([128, 2048], dt.float32, name="ps_w", tag="ifog")
            for k in range(4):
                nc.tensor.matmul(ps_w[:, k * 512:k * 512 + BC], wb_sc[k][0:1, :],
                                 wk[0:1, k * 256:k * 256 + BC], start=True, stop=True,
                                 skip_group_check=True)
            for k in range(4):
                nc.vector.tensor_tensor(out=prod[:, 0:BC],
                                        in0=ps_w[:, k * 512:k * 512 + BC],
                                        in1=esrc[k], op=OP.mult)
                if k == 0:
                    nc.vector.tensor_copy(out=facc[:, 0:BC], in_=prod[:, 0:BC])
                else:
                    nc.vector.tensor_tensor(out=facc[:, 0:BC], in0=facc[:, 0:BC],
                                            in1=prod[:, 0:BC], op=OP.add)
            ps_t = ppool.tile([128, 2048], dt.float32, name="ps_t", tag="ifog")
            ot0 = const.tile([128, 128], dt.float32, name="ot0", tag="ot0")
            nc.tensor.transpose(ps_t[:, 0:128], facc[:, 0:128], ident[:])
            nc.vector.tensor_copy(out=ot0[:], in_=ps_t[:, 0:128])
            nc.sync.dma_start(out_d[0:128, :], ot0[:])
            ot1 = const.tile([128, 128], dt.float32, name="ot1", tag="ot1")
            nc.tensor.transpose(ps_t[0:BC - 128, 512:640], facc[:, 128:BC], ident[:])
            nc.vector.tensor_copy(out=ot1[0:BC - 128, :], in_=ps_t[0:BC - 128, 512:640])
            nc.sync.dma_start(out_d[128:BC, :], ot1[0:BC - 128, :])

    nc.compile()
    return nc


# =========================== host side ===========================

def _pack_lstm(p, prescale=1.0):
    """torch-gate-order params -> (wih [128,512], whh [128,512], brow [1,512]).

    col layout per gate g in order [i, f, o, g(cell)]: fwd lhsT cols 0:64,
    bwd 64:128. whh is block-diag fwd/bwd, pre-transposed for lhsT use.
    """
    rows = {0: slice(0, 64), 1: slice(64, 128), 2: slice(192, 256), 3: slice(128, 192)}
    wih = np.zeros((128, 512), F32)
    whh = np.zeros((128, 512), F32)
    brow = np.zeros((1, 512), F32)
    for g in range(4):
        r = rows[g]
        wih[:, g * 128:g * 128 + 64] = np.asarray(p["Wih_f"], F32)[r].T * prescale
        wih[:, g * 128 + 64:(g + 1) * 128] = np.asarray(p["Wih_b"], F32)[r].T * prescale
        whh[0:64, g * 128:g * 128 + 64] = np.asarray(p["Whh_f"], F32)[r].T
        whh[64:128, g * 128 + 64:(g + 1) * 128] = np.asarray(p["Whh_b"], F32)[r].T
        brow[0, g * 128:g * 128 + 64] = np.asarray(p["b_f"], F32)[r]
        brow[0, g * 128 + 64:(g + 1) * 128] = np.asarray(p["b_b"], F32)[r]
    return wih, whh, brow


def _ids_to_tiles(ids, rows):
    full = np.zeros(rows, np.int64)
    full[:len(ids)] = np.asarray(ids, np.int64)
    return np.ascontiguousarray(full.reshape(rows // 128, 128).T.astype(np.int32))


def make_in_maps(feats, rnn_a_content, rnn_p_content, rnn_v_content,
                 rnn_a_neigh, rnn_p_neigh, rnn_v_neigh, a_neigh_att,
                 id_batch, a_neigh_ids, p_neigh_ids, v_neigh_ids):
    f = {k: np.asarray(v, F32) for k, v in feats.items()}
    a_tab = np.ascontiguousarray(np.concatenate(
        [f["a_coop"], f["a_net"], f["a_text"]], axis=1))
    p_tab = np.ascontiguousarray(np.concatenate(
        [f["p_t"], f["p_v_net"], f["p_a_net"], f["p_net"]], axis=1))
    v_tab = np.ascontiguousarray(np.concatenate(
        [f["v_net"], f["v_text"]], axis=1))

    packs = [
        _pack_lstm(rnn_a_content), _pack_lstm(rnn_p_content), _pack_lstm(rnn_v_content),
        _pack_lstm(rnn_a_neigh, 1.0 / TS_A), _pack_lstm(rnn_p_neigh, 1.0 / TS_P),
        _pack_lstm(rnn_v_neigh, 1.0 / TS_V),
    ]
    wih = np.concatenate([p[0] for p in packs], axis=1).astype(BF16)
    whh = np.concatenate([p[1] for p in packs], axis=1).astype(BF16)
    brow = np.concatenate([p[2] for p in packs], axis=1).astype(BF16)
    # per-partition bias vectors [128, 6*4]: col l*4+g = [b_fwd(64); b_bwd(64)]
    bvec = np.zeros((128, 24), F32)
    for l in range(6):
        br = packs[l][2][0]
        for g in range(4):
            bvec[0:64, l * 4 + g] = br[g * 128:g * 128 + 64]
            bvec[64:128, l * 4 + g] = br[g * 128 + 64:(g + 1) * 128]

    att = np.asarray(a_neigh_att, F32)          # [2d, 1]
    attA, attB = att[0:128, 0], att[128:256, 0]
    attw = np.stack([attA / TS_A, attB / TS_A, attB / A_NB, attB / P_NB,
                     attB / V_NB], axis=1).astype(F32)     # [128, 5]

    idb = np.asarray(id_batch, np.int64)
    anb = np.asarray(a_neigh_ids, np.int64)
    pnb = np.asarray(p_neigh_ids, np.int64)
    vnb = np.asarray(v_neigh_ids, np.int64)

    in_maps = []
    for c in range(NCORES):
        s = slice(c * BC, (c + 1) * BC)
        ids_a = np.concatenate([idb[s], anb[s].T.ravel()])   # slot-major (t-major)
        in_maps.append({
            "a_tab": a_tab, "p_tab": p_tab, "v_tab": v_tab,
            "idx_a": _ids_to_tiles(ids_a, ROWS_A),
            "idx_p": _ids_to_tiles(pnb[s].T.ravel(), ROWS_P),
            "idx_v": _ids_to_tiles(vnb[s].T.ravel(), ROWS_V),
            "wih_d": wih, "whh_d": whh, "brow_d": brow, "bvec_d": bvec.copy(),
            "att_d": attw,
        })
    return in_maps


def get_program():
    if "nc" not in _CACHE:
        _CACHE["nc"] = _build_program()
    return _CACHE["nc"]


def kernel(**inputs):
    from concourse import bass_utils

    nc = get_program()
    in_maps = make_in_maps(**inputs)
    res = bass_utils.run_bass_kernel_spmd(nc, in_maps, core_ids=list(range(NCORES)))
    out = np.concatenate([res.results[c]["out_d"] for c in range(NCORES)], axis=0)
    return np.asarray(out, F32)


# revision 31
# speedup vs baseline: 1.7398x; 1.4691x over previous
"""ARMHGNN heterogeneous-GNN message-passing kernel for 8x TRN2 NeuronCores.

Data-parallel over the node batch (250 nodes/core). Feature tables replicated;
gathers done on-device via indirect DMA; content+neighbor BiLSTMs run in a
[feature-on-partition, batch-on-free] layout with partition-packed fwd/bwd
gates (fwd in partitions 0:64, bwd 64:128); recurrent matmuls use one
block-diagonal K=128 matmul per gate; biases enter PSUM via K=1 matmuls so
activations need no per-gate bias and can batch I/F/O in one sigmoid op.
Gathered rows are cast to bf16, staged to DRAM, and transposed into
[feature, row] layout by the DMA xbar (no PE transposes on the hot path).
"""

import sys

sys.path.insert(0, "/opt/trn_rl_repo")

import numpy as np
import ml_dtypes

# ---------------- problem constants (hardcoded per contract) ----------------
D = 128          # embed dim
H = 64           # LSTM hidden per direction
BATCH = 2000
NCORES = 8
BC = BATCH // NCORES          # 250 nodes per core
A_NB, P_NB, V_NB = 10, 20, 3
A_N, P_N, V_N = 100000, 200000, 1000

RE_A, RE_P, RE_V = 5 * D, 4 * D, 6 * D          # concat row elems: 640/512/768
TS_A, TS_P, TS_V = 5, 4, 6                       # content seq lens
ROWS_A = 2816    # 250 + 2500 -> pad to 22*128
ROWS_P = 5120    # 5000 -> 40*128
ROWS_V = 768     # 750 -> 6*128
NG_A, NG_P, NG_V = ROWS_A // 128, ROWS_P // 128, ROWS_V // 128   # gather tiles

# lstm index order
L_AC, L_PC, L_VC, L_AN, L_PN, L_VN = range(6)

F32 = np.float32
BF16 = ml_dtypes.bfloat16

_CACHE = {}


def _tiles_of(lo, hi):
    """[(col_offset, n_cols), ...] <=512-wide tiles covering [lo, hi)."""
    out = []
    c = lo
    while c < hi:
        n = min(512, hi - c)
        out.append((c, n))
        c += n
    return out


def _build_program():
    import concourse.bass as bass
    import concourse.tile as tile
    from concourse import bacc, mybir
    from concourse.masks import make_identity
    import contextlib

    dt = mybir.dt
    AF = mybir.ActivationFunctionType
    OP = mybir.AluOpType

    nc = bacc.Bacc("TRN2", target_bir_lowering=False, debug=False,
                   enable_asserts=False, num_devices=NCORES)

    # ---- dram io ----
    a_tab = nc.dram_tensor("a_tab", [A_N, RE_A], dt.float32, kind="ExternalInput").ap()
    p_tab = nc.dram_tensor("p_tab", [P_N, RE_P], dt.float32, kind="ExternalInput").ap()
    v_tab = nc.dram_tensor("v_tab", [V_N, RE_V], dt.float32, kind="ExternalInput").ap()
    idx_a = nc.dram_tensor("idx_a", [128, NG_A], dt.int32, kind="ExternalInput").ap()
    idx_p = nc.dram_tensor("idx_p", [128, NG_P], dt.int32, kind="ExternalInput").ap()
    idx_v = nc.dram_tensor("idx_v", [128, NG_V], dt.int32, kind="ExternalInput").ap()
    wih_d = nc.dram_tensor("wih_d", [128, 6 * 512], dt.bfloat16, kind="ExternalInput").ap()
    whh_d = nc.dram_tensor("whh_d", [128, 6 * 512], dt.bfloat16, kind="ExternalInput").ap()
    brow_d = nc.dram_tensor("brow_d", [1, 6 * 512], dt.bfloat16, kind="ExternalInput").ap()
    bvec_d = nc.dram_tensor("bvec_d", [128, 24], dt.float32, kind="ExternalInput").ap()
    att_d = nc.dram_tensor("att_d", [128, 5], dt.float32, kind="ExternalInput").ap()
    out_d = nc.dram_tensor("out_d", [BC, D], dt.float32, kind="ExternalOutput").ap()

    with tile.TileContext(nc) as tc:
        ctx = contextlib.ExitStack()
        with ctx:
            const = ctx.enter_context(tc.tile_pool(name="const", bufs=1))
            dramp = ctx.enter_context(tc.tile_pool(name="dramp", bufs=1, space="DRAM"))
            gpool = ctx.enter_context(tc.tile_pool(name="gpool", bufs=3))
            xpool = ctx.enter_context(tc.tile_pool(name="xpool", bufs=1))
            spool = ctx.enter_context(tc.tile_pool(name="spool", bufs=2))
            hcpool = ctx.enter_context(tc.tile_pool(name="hcpool", bufs=8))
            apool = ctx.enter_context(tc.tile_pool(name="apool", bufs=1))
            ppool = ctx.enter_context(tc.tile_pool(name="ppool", bufs=2, space="PSUM"))

            # ---- constants into sbuf ----
            wih = const.tile([128, 6 * 512], dt.bfloat16, name="wih", tag="wih")
            nc.sync.dma_start(wih[:], wih_d[:])
            whh = const.tile([128, 6 * 512], dt.bfloat16, name="whh", tag="whh")
            nc.sync.dma_start(whh[:], whh_d[:])
            brow = const.tile([1, 6 * 512], dt.bfloat16, name="brow", tag="brow")
            nc.sync.dma_start(brow[:], brow_d[:])
            bvec = const.tile([128, 24], dt.float32, name="bvec", tag="bvec")
            nc.sync.dma_start(bvec[:], bvec_d[:])
            attw = const.tile([128, 5], dt.float32, name="attw", tag="attw")
            nc.sync.dma_start(attw[:], att_d[:])
            ia = const.tile([128, NG_A], dt.int32, name="ia", tag="ia")
            nc.sync.dma_start(ia[:], idx_a[:])
            ip = const.tile([128, NG_P], dt.int32, name="ip", tag="ip")
            nc.sync.dma_start(ip[:], idx_p[:])
            iv = const.tile([128, NG_V], dt.int32, name="iv", tag="iv")
            nc.sync.dma_start(iv[:], idx_v[:])
            ones_row = const.tile([1, 512], dt.bfloat16, name="ones_row", tag="ones_row")
            nc.gpsimd.memset(ones_row[:], 1.0)
            ident = const.tile([128, 128], dt.float32, name="ident", tag="ident")
            make_identity(nc, ident[:])
            wb_sc = []
            for k, tsc in enumerate([TS_A, A_NB, P_NB, V_NB]):
                t_ = const.tile([1, 128], dt.float32, name=f"wbsc{k}", tag=f"wbsc{k}")
                nc.gpsimd.memset(t_[:], 1.0 / tsc)
                wb_sc.append(t_)

            # ---- DRAM staging: one tensor per (type, wave) so transpose
            # loads only depend on their own wave's stores ----
            stage = {}

            def _pair_waves(rows, k=2):
                """waves of up to 2k tiles taken from both ends inward, so the
                bidirectional neighbor LSTM's input groups finish early."""
                tiles = _tiles_of(0, rows)
                waves = []
                lo, hi = 0, len(tiles)
                while lo < hi:
                    kk = min(k, (hi - lo + 1) // 2)
                    w = tiles[lo:lo + kk] + tiles[max(lo + kk, hi - kk):hi]
                    waves.append(w)
                    lo += kk
                    hi = max(lo, hi - kk)
                return waves

            TYPES = {
                "v": dict(tab=v_tab, idx=iv, ng=NG_V, re=RE_V, rows=ROWS_V,
                          ts=TS_V, lc=L_VC, waves=_pair_waves(ROWS_V),
                          xslot=TS_V * 768, xbufs=1),
                "a": dict(tab=a_tab, idx=ia, ng=NG_A, re=RE_A, rows=ROWS_A,
                          ts=TS_A, lc=L_AC, waves=_pair_waves(ROWS_A),
                          xslot=TS_A * 1792, xbufs=2),
                "p": dict(tab=p_tab, idx=ip, ng=NG_P, re=RE_P, rows=ROWS_P,
                          ts=TS_P, lc=L_PC, waves=_pair_waves(ROWS_P),
                          xslot=TS_P * 2048, xbufs=2),
            }

            # ---- phase 1: gather (cast f32->bf16 in-flight) + stage ----
            # Emitted in wave order (front/back paired tiles) so each wave's
            # staging completes as early as possible; types in consume order.
            for ty in ("v", "a", "p"):
                m = TYPES[ty]
                for w, chunks in enumerate(m["waves"]):
                    wrows = sum(N for _, N in chunks)
                    st = dramp.tile([wrows, m["re"]], dt.bfloat16,
                                    name=f"stage_{ty}{w}", tag=f"st_{ty}{w}")
                    stage[(ty, w)] = st
                    loc = 0
                    for co, N in chunks:
                        for j in range(co // 128, (co + N) // 128):
                            gt = gpool.tile([128, m["re"]], dt.bfloat16,
                                            name=f"g_{ty}{j}", tag="g",
                                            padded_shape=[128, RE_V])
                            nc.gpsimd.indirect_dma_start(
                                out=gt[:], out_offset=None, in_=m["tab"][:],
                                in_offset=bass.IndirectOffsetOnAxis(
                                    ap=m["idx"][:, j:j + 1], axis=0),
                            )
                            o = loc + j * 128 - co
                            nc.gpsimd.dma_start(out=st[o:o + 128, :], in_=gt[:])
                        loc += N

            # ---- content state/accumulators ----
            # hacc_a0: f32 c_agg slice (attention); other content sums in bf16
            hacc_a0 = apool.tile([128, 256], dt.float32, name="hacc_a0", tag="hacc_a0")
            hacc_nbp = {
                "a": apool.tile([128, ROWS_A - 250], dt.bfloat16, name="hacc_an", tag="hacc_an"),
                "p": apool.tile([128, ROWS_P], dt.bfloat16, name="hacc_pn", tag="hacc_pn"),
                "v": apool.tile([128, ROWS_V], dt.bfloat16, name="hacc_vn", tag="hacc_vn"),
            }

            def hacc_segs(ty, co, N):
                """content hacc write segments: [(ap, off, n, first_is_copy)]"""
                if ty != "a":
                    return [(hacc_nbp[ty], co, N)]
                segs = []
                if co < 250:
                    n0 = min(N, 250 - co)
                    segs.append((hacc_a0, co, n0))
                    if N > n0:
                        segs.append((hacc_nbp["a"], 0, N - n0))
                else:
                    segs.append((hacc_nbp["a"], co - 250, N))
                return segs

            # ---------------- LSTM step helper ----------------
            def lstm_step(l, t, xf, xb, h_t, c_t, segs, N, tag):
                ps = ppool.tile([128, 2048], dt.float32, name=f"ps_{tag}", tag="ifog")
                first = t == 0
                for g in range(4):
                    gs = g * 512
                    nc.tensor.matmul(ps[0:64, gs:gs + N],
                                     wih[:, l * 512 + g * 128: l * 512 + g * 128 + 64],
                                     xf, start=True, stop=False, skip_group_check=True)
                    nc.tensor.matmul(ps[64:128, gs:gs + N],
                                     wih[:, l * 512 + g * 128 + 64: l * 512 + (g + 1) * 128],
                                     xb, start=True, stop=first and g == 3,
                                     skip_group_check=True)
                for g in range(3):
                    gs = g * 512
                    nc.tensor.matmul(ps[:, gs:gs + N],
                                     brow[0:1, l * 512 + g * 128: l * 512 + (g + 1) * 128],
                                     ones_row[0:1, 0:N], start=False, stop=first,
                                     skip_group_check=True)
                # recurrent matmuls last: they depend on h(t-1), so issuing
                # them after the x/bias matmuls keeps PE fed while h settles
                if not first:
                    for g in range(4):
                        gs = g * 512
                        nc.tensor.matmul(ps[:, gs:gs + N],
                                         whh[:, l * 512 + g * 128: l * 512 + (g + 1) * 128],
                                         h_t[:, 0:N], start=False, stop=True,
                                         skip_group_check=True)
                sig = spool.tile([128, 1536], dt.bfloat16, name=f"sig_{tag}", tag="sig")
                ps_ifo = ps[:, 0:1536].rearrange("p (g n) -> p g n", g=3)[:, :, 0:N]
                sg_ifo = sig[:, 0:3 * N].rearrange("p (g n) -> p g n", g=3)
                nc.scalar.activation(sg_ifo, ps_ifo, AF.Sigmoid)
                gt_ = spool.tile([128, 512], dt.bfloat16, name=f"gt_{tag}", tag="gt")
                nc.scalar.activation(gt_[:, 0:N], ps[:, 1536:1536 + N], AF.Tanh,
                                     bias=bvec[:, l * 4 + 3: l * 4 + 4])
                sI, sF, sO = sig[:, 0:N], sig[:, N:2 * N], sig[:, 2 * N:3 * N]
                if first:
                    nc.vector.tensor_tensor(out=c_t[:, 0:N], in0=sI, in1=gt_[:, 0:N],
                                            op=OP.mult)
                else:
                    tm1 = spool.tile([128, 512], dt.float32, name=f"tm1_{tag}", tag="tm1")
                    nc.vector.tensor_tensor(out=tm1[:, 0:N], in0=sF, in1=c_t[:, 0:N],
                                            op=OP.mult)
                    tm2 = spool.tile([128, 512], dt.bfloat16, name=f"tm2_{tag}", tag="tm2")
                    nc.vector.tensor_tensor(out=tm2[:, 0:N], in0=sI, in1=gt_[:, 0:N],
                                            op=OP.mult)
                    nc.vector.tensor_tensor(out=c_t[:, 0:N], in0=tm1[:, 0:N],
                                            in1=tm2[:, 0:N], op=OP.add)
                tc_ = spool.tile([128, 512], dt.bfloat16, name=f"tc_{tag}", tag="tc")
                nc.scalar.activation(tc_[:, 0:N], c_t[:, 0:N], AF.Tanh)
                nc.vector.tensor_tensor(out=h_t[:, 0:N], in0=sO, in1=tc_[:, 0:N],
                                        op=OP.mult)
                loc = 0
                for hap, off, n in segs:
                    if first:
                        nc.vector.tensor_copy(out=hap[:, off:off + n],
                                              in_=h_t[:, loc:loc + n])
                    else:
                        nc.vector.tensor_tensor(out=hap[:, off:off + n],
                                                in0=hap[:, off:off + n],
                                                in1=h_t[:, loc:loc + n], op=OP.add)
                    loc += n

            # ---------------- content + neighbor LSTMs, interleaved ----------------
            # Tile-major content chains (all T steps per tile) emitted
            # round-robin across types, with neighbor-LSTM steps interleaved
            # as soon as their input group's columns are fully accumulated.
            # This removes the serial neighbor tail and keeps PE dense.
            NB = {
                "a": dict(ln=L_AN, tnb=A_NB, goff=250),
                "p": dict(ln=L_PN, tnb=P_NB, goff=0),
                "v": dict(ln=L_VN, tnb=V_NB, goff=0),
            }
            hacc_nb = {}
            nbst = {}
            for ty in ("v", "a", "p"):
                hacc_nb[ty] = apool.tile([128, 256], dt.float32,
                                         name=f"haccnb_{ty}", tag=f"haccnb_{ty}")
                nbst[ty] = dict(
                    hn=hcpool.tile([128, 512], dt.bfloat16, name=f"hn_{ty}", tag="h"),
                    cn=hcpool.tile([128, 512], dt.float32, name=f"cn_{ty}", tag="c"))

            def emit_nb_step(ty, t):
                mb = NB[ty]
                src = hacc_nbp[ty]
                rt = mb["tnb"] - 1 - t
                lstm_step(mb["ln"], t,
                          src[:, t * BC:(t + 1) * BC],
                          src[:, rt * BC:(rt + 1) * BC],
                          nbst[ty]["hn"], nbst[ty]["cn"],
                          [(hacc_nb[ty], 0, BC)], BC, f"n{ty}{t}")

            # per-type work lists: each wave pairs a front tile and a back
            # tile so both ends of the group sequence finish early (the
            # bidirectional neighbor LSTM reads group t AND group tnb-1-t
            # at step t). Emission order must respect dataflow: a neighbor
            # step is emitted only once every column of both its groups has
            # been written by an emitted content chain.
            def build_seq(ty):
                m = TYPES[ty]
                seq = []
                for w in range(len(m["waves"])):
                    seq.append(("loads", w))
                    seq.append(("wave", w))
                return seq

            seqs = {ty: build_seq(ty) for ty in ("v", "a", "p")}
            xw_cur = {ty: None for ty in seqs}
            covered = {ty: np.zeros(TYPES[ty]["rows"], bool) for ty in seqs}
            nb_next = {ty: 0 for ty in seqs}

            def nb_ready(ty, t):
                mb = NB[ty]
                g0 = mb["goff"] + t * BC
                g1 = mb["goff"] + (mb["tnb"] - 1 - t) * BC
                cv = covered[ty]
                return cv[g0:g0 + BC].all() and cv[g1:g1 + BC].all()

            def emit_item(ty, item):
                m = TYPES[ty]
                if item[0] == "loads":
                    # one whole-row xbar-transpose DMA per wave: linear read
                    # of [wrows, re] -> [128, ts, wrows] (all timesteps)
                    w = item[1]
                    chunks = m["waves"][w]
                    st = stage[(ty, w)]
                    wrows = sum(N for _, N in chunks)
                    xt_t = xpool.tile([128, m["xslot"]], dt.bfloat16,
                                      name=f"xt_{ty}_{w}", tag=f"xt_{ty}",
                                      bufs=m["xbufs"])
                    v3 = xt_t[:, 0:m["ts"] * wrows].rearrange(
                        "p (t r) -> p t r", t=m["ts"])
                    nc.sync.dma_start(v3, st[0:wrows, :], transpose=True)
                    xw_cur[ty] = (w, v3)
                else:
                    # t-major over the whole wave: all chunks' step-t matmuls
                    # issue together, so the in-order PE queue never
                    # head-of-line-blocks on a single chain's recurrence
                    w = item[1]
                    w2, xw = xw_cur[ty]
                    assert w2 == w
                    chunks = m["waves"][w]
                    locs = []
                    loc = 0
                    for co, N in chunks:
                        locs.append(loc)
                        loc += N
                    hts = [hcpool.tile([128, 512], dt.bfloat16,
                                       name=f"h_{ty}_{co}", tag="h")
                           for co, N in chunks]
                    cts = [hcpool.tile([128, 512], dt.float32,
                                       name=f"c_{ty}_{co}", tag="c")
                           for co, N in chunks]
                    rts = m["ts"]
                    for t in range(rts):
                        for i, (co, N) in enumerate(chunks):
                            lstm_step(m["lc"], t,
                                      xw[:, t, locs[i]:locs[i] + N],
                                      xw[:, rts - 1 - t, locs[i]:locs[i] + N],
                                      hts[i], cts[i], hacc_segs(ty, co, N), N,
                                      f"c{ty}{t}_{co}")
                    for co, N in chunks:
                        covered[ty][co:co + N] = True
                    while (nb_next[ty] < NB[ty]["tnb"]
                           and nb_ready(ty, nb_next[ty])):
                        emit_nb_step(ty, nb_next[ty])
                        nb_next[ty] += 1

            # type-sequential emission (feed order); neighbor steps interleave
            # via emit_item's readiness checks
            for ty in ("v", "a", "p"):
                for item in seqs[ty]:
                    emit_item(ty, item)

            # ---------------- attention combine ----------------
            srcs = [hacc_a0[:, 0:BC], hacc_a0[:, 0:BC], hacc_nb["a"][:, 0:BC],
                    hacc_nb["p"][:, 0:BC], hacc_nb["v"][:, 0:BC]]
            ps_s = ppool.tile([128, 2048], dt.float32, name="ps_s", tag="ifog")
            for k in range(5):
                nc.tensor.matmul(ps_s[0:1, k * 256:k * 256 + BC], attw[:, k:k + 1],
                                 srcs[k], start=True, stop=True, skip_group_check=True)
            sb_s = const.tile([1, 5 * 256], dt.float32, name="sb_s", tag="sb_s")
            for k in range(5):
                nc.vector.tensor_copy(out=sb_s[0:1, k * 256:k * 256 + BC],
                                      in_=ps_s[0:1, k * 256:k * 256 + BC])
            lr = const.tile([1, 4 * 256], dt.float32, name="lr", tag="lr")
            nc.gpsimd.memset(lr[:], 0.0)
            for k in range(4):
                nc.vector.tensor_tensor(out=lr[0:1, k * 256:k * 256 + BC],
                                        in0=sb_s[0:1, 0:BC],
                                        in1=sb_s[0:1, (k + 1) * 256:(k + 1) * 256 + BC],
                                        op=OP.add)
            lr2 = const.tile([1, 4 * 256], dt.float32, name="lr2", tag="lr2")
            nc.vector.tensor_scalar_mul(lr2[:], lr[:], 0.01)
            nc.vector.tensor_tensor(out=lr2[:], in0=lr2[:], in1=lr[:], op=OP.max)
            ex = const.tile([1, 4 * 256], dt.float32, name="ex", tag="ex")
            nc.scalar.activation(ex[:], lr2[:], AF.Exp)
            zz = const.tile([1, 256], dt.float32, name="zz", tag="zz")
            nc.vector.tensor_tensor(out=zz[0:1, 0:BC], in0=ex[0:1, 0:BC],
                                    in1=ex[0:1, 256:256 + BC], op=OP.add)
            nc.vector.tensor_tensor(out=zz[0:1, 0:BC], in0=zz[0:1, 0:BC],
                                    in1=ex[0:1, 512:512 + BC], op=OP.add)
            nc.vector.tensor_tensor(out=zz[0:1, 0:BC], in0=zz[0:1, 0:BC],
                                    in1=ex[0:1, 768:768 + BC], op=OP.add)
            rz = const.tile([1, 256], dt.float32, name="rz", tag="rz")
            nc.vector.reciprocal(rz[0:1, 0:BC], zz[0:1, 0:BC])
            wk = const.tile([1, 4 * 256], dt.float32, name="wk", tag="wk")
            for k in range(4):
                nc.vector.tensor_tensor(out=wk[0:1, k * 256:k * 256 + BC],
                                        in0=ex[0:1, k * 256:k * 256 + BC],
                                        in1=rz[0:1, 0:BC], op=OP.mult)
            esrc = [hacc_a0[:, 0:BC], hacc_nb["a"][:, 0:BC],
                    hacc_nb["p"][:, 0:BC], hacc_nb["v"][:, 0:BC]]
            facc = const.tile([128, 256], dt.float32, name="facc", tag="facc")
            prod = const.tile([128, 256], dt.float32, name="prod", tag="prod")
            ps_w = ppool.tile([128, 2048], dt.float32, name="ps_w", tag="ifog")
            for k in range(4):
                nc.tensor.matmul(ps_w[:, k * 512:k * 512 + BC], wb_sc[k][0:1, :],
                                 wk[0:1, k * 256:k * 256 + BC], start=True, stop=True,
                                 skip_group_check=True)
            for k in range(4):
                nc.vector.tensor_tensor(out=prod[:, 0:BC],
                                        in0=ps_w[:, k * 512:k * 512 + BC],
                                        in1=esrc[k], op=OP.mult)
                if k == 0:
                    nc.vector.tensor_copy(out=facc[:, 0:BC], in_=prod[:, 0:BC])
                else:
                    nc.vector.tensor_tensor(out=facc[:, 0:BC], in0=facc[:, 0:BC],
                                            in1=prod[:, 0:BC], op=OP.add)
            ps_t = ppool.tile([128, 2048], dt.float32, name="ps_t", tag="ifog")
            ot0 = const.tile([128, 128], dt.float32, name="ot0", tag="ot0")
            nc.tensor.transpose(ps_t[:, 0:128], facc[:, 0:128], ident[:])
            nc.vector.tensor_copy(out=ot0[:], in_=ps_t[:, 0:128])
            nc.sync.dma_start(out_d[0:128, :], ot0[:])
            ot1 = const.tile([128, 128], dt.float32, name="ot1", tag="ot1")
            nc.tensor.transpose(ps_t[0:BC - 128, 512:640], facc[:, 128:BC], ident[:])
            nc.vector.tensor_copy(out=ot1[0:BC - 128, :], in_=ps_t[0:BC - 128, 512:640])
            nc.sync.dma_start(out_d[128:BC, :], ot1[0:BC - 128, :])

    nc.compile()
    return nc


# =========================== host side ===========================

def _pack_lstm(p, prescale=1.0):
    """torch-gate-order params -> (wih [128,512], whh [128,512], brow [1,512]).

    col layout per gate g in order [i, f, o, g(cell)]: fwd lhsT cols 0:64,
    bwd 64:128. whh is block-diag fwd/bwd, pre-transposed for lhsT use.
    """
    rows = {0: slice(0, 64), 1: slice(64, 128), 2: slice(192, 256), 3: slice(128, 192)}
    wih = np.zeros((128, 512), F32)
    whh = np.zeros((128, 512), F32)
    brow = np.zeros((1, 512), F32)
    for g in range(4):
        r = rows[g]
        wih[:, g * 128:g * 128 + 64] = np.asarray(p["Wih_f"], F32)[r].T * prescale
        wih[:, g * 128 + 64:(g + 1) * 128] = np.asarray(p["Wih_b"], F32)[r].T * prescale
        whh[0:64, g * 128:g * 128 + 64] = np.asarray(p["Whh_f"], F32)[r].T
        whh[64:128, g * 128 + 64:(g + 1) * 128] = np.asarray(p["Whh_b"], F32)[r].T
        brow[0, g * 128:g * 128 + 64] = np.asarray(p["b_f"], F32)[r]
        brow[0, g * 128 + 64:(g + 1) * 128] = np.asarray(p["b_b"], F32)[r]
    return wih, whh, brow


def _ids_to_tiles(ids, rows):
    full = np.zeros(rows, np.int64)
    full[:len(ids)] = np.asarray(ids, np.int64)
    return np.ascontiguousarray(full.reshape(rows // 128, 128).T.astype(np.int32))


def make_in_maps(feats, rnn_a_content, rnn_p_content, rnn_v_content,
                 rnn_a_neigh, rnn_p_neigh, rnn_v_neigh, a_neigh_att,
                 id_batch, a_neigh_ids, p_neigh_ids, v_neigh_ids):
    f = {k: np.asarray(v, F32) for k, v in feats.items()}
    a_tab = np.ascontiguousarray(np.concatenate(
        [f["a_coop"], f["a_net"], f["a_text"]], axis=1))
    p_tab = np.ascontiguousarray(np.concatenate(
        [f["p_t"], f["p_v_net"], f["p_a_net"], f["p_net"]], axis=1))
    v_tab = np.ascontiguousarray(np.concatenate(
        [f["v_net"], f["v_text"]], axis=1))

    packs = [
        _pack_lstm(rnn_a_content), _pack_lstm(rnn_p_content), _pack_lstm(rnn_v_content),
        _pack_lstm(rnn_a_neigh, 1.0 / TS_A), _pack_lstm(rnn_p_neigh, 1.0 / TS_P),
        _pack_lstm(rnn_v_neigh, 1.0 / TS_V),
    ]
    wih = np.concatenate([p[0] for p in packs], axis=1).astype(BF16)
    whh = np.concatenate([p[1] for p in packs], axis=1).astype(BF16)
    brow = np.concatenate([p[2] for p in packs], axis=1).astype(BF16)
    # per-partition bias vectors [128, 6*4]: col l*4+g = [b_fwd(64); b_bwd(64)]
    bvec = np.zeros((128, 24), F32)
    for l in range(6):
        br = packs[l][2][0]
        for g in range(4):
            bvec[0:64, l * 4 + g] = br[g * 128:g * 128 + 64]
            bvec[64:128, l * 4 + g] = br[g * 128 + 64:(g + 1) * 128]

    att = np.asarray(a_neigh_att, F32)          # [2d, 1]
    attA, attB = att[0:128, 0], att[128:256, 0]
    attw = np.stack([attA / TS_A, attB / TS_A, attB / A_NB, attB / P_NB,
                     attB / V_NB], axis=1).astype(F32)     # [128, 5]

    idb = np.asarray(id_batch, np.int64)
    anb = np.asarray(a_neigh_ids, np.int64)
    pnb = np.asarray(p_neigh_ids, np.int64)
    vnb = np.asarray(v_neigh_ids, np.int64)

    in_maps = []
    for c in range(NCORES):
        s = slice(c * BC, (c + 1) * BC)
        ids_a = np.concatenate([idb[s], anb[s].T.ravel()])   # slot-major (t-major)
        in_maps.append({
            "a_tab": a_tab, "p_tab": p_tab, "v_tab": v_tab,
            "idx_a": _ids_to_tiles(ids_a, ROWS_A),
            "idx_p": _ids_to_tiles(pnb[s].T.ravel(), ROWS_P),
            "idx_v": _ids_to_tiles(vnb[s].T.ravel(), ROWS_V),
            "wih_d": wih, "whh_d": whh, "brow_d": brow, "bvec_d": bvec.copy(),
            "att_d": attw,
        })
    return in_maps


def get_program():
    if "nc" not in _CACHE:
        _CACHE["nc"] = _build_program()
    return _CACHE["nc"]


def kernel(**inputs):
    from concourse import bass_utils

    nc = get_program()
    in_maps = make_in_maps(**inputs)
    res = bass_utils.run_bass_kernel_spmd(nc, in_maps, core_ids=list(range(NCORES)))
    out = np.concatenate([res.results[c]["out_d"] for c in range(NCORES)], axis=0)
    return np.asarray(out, F32)


# revision 36
# speedup vs baseline: 1.7422x; 1.0014x over previous
"""ARMHGNN heterogeneous-GNN message-passing kernel for 8x TRN2 NeuronCores.

Data-parallel over the node batch (250 nodes/core). Feature tables replicated;
gathers done on-device via indirect DMA; content+neighbor BiLSTMs run in a
[feature-on-partition, batch-on-free] layout with partition-packed fwd/bwd
gates (fwd in partitions 0:64, bwd 64:128); recurrent matmuls use one
block-diagonal K=128 matmul per gate; biases enter PSUM via K=1 matmuls so
activations need no per-gate bias and can batch I/F/O in one sigmoid op.
Gathered rows are cast to bf16, staged to DRAM, and transposed into
[feature, row] layout by the DMA xbar (no PE transposes on the hot path).
"""

import sys

sys.path.insert(0, "/opt/trn_rl_repo")

import numpy as np
import ml_dtypes

# ---------------- problem constants (hardcoded per contract) ----------------
D = 128          # embed dim
H = 64           # LSTM hidden per direction
BATCH = 2000
NCORES = 8
BC = BATCH // NCORES          # 250 nodes per core
A_NB, P_NB, V_NB = 10, 20, 3
A_N, P_N, V_N = 100000, 200000, 1000

RE_A, RE_P, RE_V = 5 * D, 4 * D, 6 * D          # concat row elems: 640/512/768
TS_A, TS_P, TS_V = 5, 4, 6                       # content seq lens
ROWS_A = 2816    # 250 + 2500 -> pad to 22*128
ROWS_P = 5120    # 5000 -> 40*128
ROWS_V = 768     # 750 -> 6*128
NG_A, NG_P, NG_V = ROWS_A // 128, ROWS_P // 128, ROWS_V // 128   # gather tiles

# lstm index order
L_AC, L_PC, L_VC, L_AN, L_PN, L_VN = range(6)

F32 = np.float32
BF16 = ml_dtypes.bfloat16

_CACHE = {}


def _tiles_of(lo, hi):
    """[(col_offset, n_cols), ...] <=512-wide tiles covering [lo, hi)."""
    out = []
    c = lo
    while c < hi:
        n = min(512, hi - c)
        out.append((c, n))
        c += n
    return out


def _build_program():
    import concourse.bass as bass
    import concourse.tile as tile
    from concourse import bacc, mybir
    from concourse.masks import make_identity
    import contextlib

    dt = mybir.dt
    AF = mybir.ActivationFunctionType
    OP = mybir.AluOpType

    nc = bacc.Bacc("TRN2", target_bir_lowering=False, debug=False,
                   enable_asserts=False, num_devices=NCORES)

    # ---- dram io ----
    a_tab = nc.dram_tensor("a_tab", [A_N, RE_A], dt.float32, kind="ExternalInput").ap()
    p_tab = nc.dram_tensor("p_tab", [P_N, RE_P], dt.float32, kind="ExternalInput").ap()
    v_tab = nc.dram_tensor("v_tab", [V_N, RE_V], dt.float32, kind="ExternalInput").ap()
    idx_a = nc.dram_tensor("idx_a", [128, NG_A], dt.int32, kind="ExternalInput").ap()
    idx_p = nc.dram_tensor("idx_p", [128, NG_P], dt.int32, kind="ExternalInput").ap()
    idx_v = nc.dram_tensor("idx_v", [128, NG_V], dt.int32, kind="ExternalInput").ap()
    wih_d = nc.dram_tensor("wih_d", [128, 6 * 512], dt.bfloat16, kind="ExternalInput").ap()
    whh_d = nc.dram_tensor("whh_d", [128, 6 * 512], dt.bfloat16, kind="ExternalInput").ap()
    brow_d = nc.dram_tensor("brow_d", [1, 6 * 512], dt.bfloat16, kind="ExternalInput").ap()
    bvec_d = nc.dram_tensor("bvec_d", [128, 24], dt.float32, kind="ExternalInput").ap()
    att_d = nc.dram_tensor("att_d", [128, 5], dt.float32, kind="ExternalInput").ap()
    out_d = nc.dram_tensor("out_d", [BC, D], dt.float32, kind="ExternalOutput").ap()

    with tile.TileContext(nc) as tc:
        ctx = contextlib.ExitStack()
        with ctx:
            const = ctx.enter_context(tc.tile_pool(name="const", bufs=1))
            dramp = ctx.enter_context(tc.tile_pool(name="dramp", bufs=1, space="DRAM"))
            gpool = ctx.enter_context(tc.tile_pool(name="gpool", bufs=3))
            xpool = ctx.enter_context(tc.tile_pool(name="xpool", bufs=1))
            spool = ctx.enter_context(tc.tile_pool(name="spool", bufs=3))
            hcpool = ctx.enter_context(tc.tile_pool(name="hcpool", bufs=8))
            apool = ctx.enter_context(tc.tile_pool(name="apool", bufs=1))
            ppool = ctx.enter_context(tc.tile_pool(name="ppool", bufs=2, space="PSUM"))

            # ---- constants into sbuf ----
            wih = const.tile([128, 6 * 512], dt.bfloat16, name="wih", tag="wih")
            nc.sync.dma_start(wih[:], wih_d[:])
            whh = const.tile([128, 6 * 512], dt.bfloat16, name="whh", tag="whh")
            nc.sync.dma_start(whh[:], whh_d[:])
            brow = const.tile([1, 6 * 512], dt.bfloat16, name="brow", tag="brow")
            nc.sync.dma_start(brow[:], brow_d[:])
            bvec = const.tile([128, 24], dt.float32, name="bvec", tag="bvec")
            nc.sync.dma_start(bvec[:], bvec_d[:])
            attw = const.tile([128, 5], dt.float32, name="attw", tag="attw")
            nc.sync.dma_start(attw[:], att_d[:])
            ia = const.tile([128, NG_A], dt.int32, name="ia", tag="ia")
            nc.sync.dma_start(ia[:], idx_a[:])
            ip = const.tile([128, NG_P], dt.int32, name="ip", tag="ip")
            nc.sync.dma_start(ip[:], idx_p[:])
            iv = const.tile([128, NG_V], dt.int32, name="iv", tag="iv")
            nc.sync.dma_start(iv[:], idx_v[:])
            ones_row = const.tile([1, 512], dt.bfloat16, name="ones_row", tag="ones_row")
            nc.gpsimd.memset(ones_row[:], 1.0)
            ident = const.tile([128, 128], dt.float32, name="ident", tag="ident")
            make_identity(nc, ident[:])
            wb_sc = []
            for k, tsc in enumerate([TS_A, A_NB, P_NB, V_NB]):
                t_ = const.tile([1, 128], dt.float32, name=f"wbsc{k}", tag=f"wbsc{k}")
                nc.gpsimd.memset(t_[:], 1.0 / tsc)
                wb_sc.append(t_)

            # ---- DRAM staging: one tensor per (type, wave) so transpose
            # loads only depend on their own wave's stores ----
            stage = {}

            def _pair_waves(rows, k=2):
                """waves of up to 2k tiles taken from both ends inward, so the
                bidirectional neighbor LSTM's input groups finish early."""
                tiles = _tiles_of(0, rows)
                waves = []
                lo, hi = 0, len(tiles)
                while lo < hi:
                    kk = min(k, (hi - lo + 1) // 2)
                    w = tiles[lo:lo + kk] + tiles[max(lo + kk, hi - kk):hi]
                    waves.append(w)
                    lo += kk
                    hi = max(lo, hi - kk)
                return waves

            TYPES = {
                "v": dict(tab=v_tab, idx=iv, ng=NG_V, re=RE_V, rows=ROWS_V,
                          ts=TS_V, lc=L_VC, waves=_pair_waves(ROWS_V),
                          xslot=TS_V * 768, xbufs=1),
                "a": dict(tab=a_tab, idx=ia, ng=NG_A, re=RE_A, rows=ROWS_A,
                          ts=TS_A, lc=L_AC, waves=_pair_waves(ROWS_A),
                          xslot=TS_A * 1792, xbufs=2),
                "p": dict(tab=p_tab, idx=ip, ng=NG_P, re=RE_P, rows=ROWS_P,
                          ts=TS_P, lc=L_PC, waves=_pair_waves(ROWS_P),
                          xslot=TS_P * 2048, xbufs=2),
            }

            # ---- phase 1: gather (cast f32->bf16 in-flight) + stage ----
            # Emitted in wave order (front/back paired tiles) so each wave's
            # staging completes as early as possible; types in consume order.
            for ty in ("v", "a", "p"):
                m = TYPES[ty]
                for w, chunks in enumerate(m["waves"]):
                    wrows = sum(N for _, N in chunks)
                    st = dramp.tile([wrows, m["re"]], dt.bfloat16,
                                    name=f"stage_{ty}{w}", tag=f"st_{ty}{w}")
                    stage[(ty, w)] = st
                    loc = 0
                    for co, N in chunks:
                        for j in range(co // 128, (co + N) // 128):
                            gt = gpool.tile([128, m["re"]], dt.bfloat16,
                                            name=f"g_{ty}{j}", tag="g",
                                            padded_shape=[128, RE_V])
                            nc.gpsimd.indirect_dma_start(
                                out=gt[:], out_offset=None, in_=m["tab"][:],
                                in_offset=bass.IndirectOffsetOnAxis(
                                    ap=m["idx"][:, j:j + 1], axis=0),
                            )
                            o = loc + j * 128 - co
                            nc.gpsimd.dma_start(out=st[o:o + 128, :], in_=gt[:])
                        loc += N

            # ---- content state/accumulators ----
            # hacc_a0: f32 c_agg slice (attention); other content sums in bf16
            hacc_a0 = apool.tile([128, 256], dt.float32, name="hacc_a0", tag="hacc_a0")
            hacc_nbp = {
                "a": apool.tile([128, ROWS_A - 250], dt.bfloat16, name="hacc_an", tag="hacc_an"),
                "p": apool.tile([128, ROWS_P], dt.bfloat16, name="hacc_pn", tag="hacc_pn"),
                "v": apool.tile([128, ROWS_V], dt.bfloat16, name="hacc_vn", tag="hacc_vn"),
            }

            def hacc_segs(ty, co, N):
                """content hacc write segments: [(ap, off, n, first_is_copy)]"""
                if ty != "a":
                    return [(hacc_nbp[ty], co, N)]
                segs = []
                if co < 250:
                    n0 = min(N, 250 - co)
                    segs.append((hacc_a0, co, n0))
                    if N > n0:
                        segs.append((hacc_nbp["a"], 0, N - n0))
                else:
                    segs.append((hacc_nbp["a"], co - 250, N))
                return segs

            # ---------------- LSTM step helpers ----------------
            # Each step is split so the c->tanh->h tail of group k is emitted
            # after group k+1's matmuls+sigmoids: the in-order ACT/DVE queues
            # then never head-of-line-block on the cell-state chain.
            pend = [None]

            def lstm_tail(st):
                sig, gt_, c_t, h_t, segs, N, first, tag = st
                sO = sig[:, 2 * N:3 * N]
                tc_ = spool.tile([128, 512], dt.bfloat16, name=f"tc_{tag}", tag="tc")
                nc.scalar.activation(tc_[:, 0:N], c_t[:, 0:N], AF.Tanh)
                nc.vector.tensor_tensor(out=h_t[:, 0:N], in0=sO, in1=tc_[:, 0:N],
                                        op=OP.mult)
                loc = 0
                for hap, off, n in segs:
                    if first:
                        nc.vector.tensor_copy(out=hap[:, off:off + n],
                                              in_=h_t[:, loc:loc + n])
                    else:
                        nc.vector.tensor_tensor(out=hap[:, off:off + n],
                                                in0=hap[:, off:off + n],
                                                in1=h_t[:, loc:loc + n], op=OP.add)
                    loc += n

            def flush_tail():
                if pend[0] is not None:
                    lstm_tail(pend[0])
                    pend[0] = None

            def lstm_step(l, t, xf, xb, h_t, c_t, segs, N, tag, pipeline=True):
                ps = ppool.tile([128, 2048], dt.float32, name=f"ps_{tag}", tag="ifog")
                first = t == 0
                for g in range(4):
                    gs = g * 512
                    nc.tensor.matmul(ps[0:64, gs:gs + N],
                                     wih[:, l * 512 + g * 128: l * 512 + g * 128 + 64],
                                     xf, start=True, stop=False, skip_group_check=True)
                    nc.tensor.matmul(ps[64:128, gs:gs + N],
                                     wih[:, l * 512 + g * 128 + 64: l * 512 + (g + 1) * 128],
                                     xb, start=True, stop=first and g == 3,
                                     skip_group_check=True)
                for g in range(3):
                    gs = g * 512
                    nc.tensor.matmul(ps[:, gs:gs + N],
                                     brow[0:1, l * 512 + g * 128: l * 512 + (g + 1) * 128],
                                     ones_row[0:1, 0:N], start=False, stop=first,
                                     skip_group_check=True)
                # recurrent matmuls last: they depend on h(t-1), so issuing
                # them after the x/bias matmuls keeps PE fed while h settles
                if not first:
                    for g in range(4):
                        gs = g * 512
                        nc.tensor.matmul(ps[:, gs:gs + N],
                                         whh[:, l * 512 + g * 128: l * 512 + (g + 1) * 128],
                                         h_t[:, 0:N], start=False, stop=True,
                                         skip_group_check=True)
                sig = spool.tile([128, 1536], dt.bfloat16, name=f"sig_{tag}", tag="sig")
                ps_ifo = ps[:, 0:1536].rearrange("p (g n) -> p g n", g=3)[:, :, 0:N]
                sg_ifo = sig[:, 0:3 * N].rearrange("p (g n) -> p g n", g=3)
                nc.scalar.activation(sg_ifo, ps_ifo, AF.Sigmoid)
                gt_ = spool.tile([128, 512], dt.bfloat16, name=f"gt_{tag}", tag="gt")
                nc.scalar.activation(gt_[:, 0:N], ps[:, 1536:1536 + N], AF.Tanh,
                                     bias=bvec[:, l * 4 + 3: l * 4 + 4])
                # previous group's tail goes to the queues here, between this
                # group's ACT heads and DVE cell-update
                prev = pend[0]
                pend[0] = None
                if prev is not None:
                    lstm_tail(prev)
                sI, sF = sig[:, 0:N], sig[:, N:2 * N]
                if first:
                    nc.vector.tensor_tensor(out=c_t[:, 0:N], in0=sI, in1=gt_[:, 0:N],
                                            op=OP.mult)
                else:
                    tm1 = spool.tile([128, 512], dt.float32, name=f"tm1_{tag}", tag="tm1")
                    nc.vector.tensor_tensor(out=tm1[:, 0:N], in0=sF, in1=c_t[:, 0:N],
                                            op=OP.mult)
                    tm2 = spool.tile([128, 512], dt.bfloat16, name=f"tm2_{tag}", tag="tm2")
                    nc.vector.tensor_tensor(out=tm2[:, 0:N], in0=sI, in1=gt_[:, 0:N],
                                            op=OP.mult)
                    nc.vector.tensor_tensor(out=c_t[:, 0:N], in0=tm1[:, 0:N],
                                            in1=tm2[:, 0:N], op=OP.add)
                st = (sig, gt_, c_t, h_t, segs, N, first, tag)
                if pipeline:
                    pend[0] = st
                else:
                    lstm_tail(st)

            # ---------------- content + neighbor LSTMs, interleaved ----------------
            # Tile-major content chains (all T steps per tile) emitted
            # round-robin across types, with neighbor-LSTM steps interleaved
            # as soon as their input group's columns are fully accumulated.
            # This removes the serial neighbor tail and keeps PE dense.
            NB = {
                "a": dict(ln=L_AN, tnb=A_NB, goff=250),
                "p": dict(ln=L_PN, tnb=P_NB, goff=0),
                "v": dict(ln=L_VN, tnb=V_NB, goff=0),
            }
            hacc_nb = {}
            nbst = {}
            for ty in ("v", "a", "p"):
                hacc_nb[ty] = apool.tile([128, 256], dt.float32,
                                         name=f"haccnb_{ty}", tag=f"haccnb_{ty}")
                nbst[ty] = dict(
                    hn=hcpool.tile([128, 512], dt.bfloat16, name=f"hn_{ty}", tag="h"),
                    cn=hcpool.tile([128, 512], dt.float32, name=f"cn_{ty}", tag="c"))

            def emit_nb_step(ty, t):
                mb = NB[ty]
                src = hacc_nbp[ty]
                rt = mb["tnb"] - 1 - t
                flush_tail()
                lstm_step(mb["ln"], t,
                          src[:, t * BC:(t + 1) * BC],
                          src[:, rt * BC:(rt + 1) * BC],
                          nbst[ty]["hn"], nbst[ty]["cn"],
                          [(hacc_nb[ty], 0, BC)], BC, f"n{ty}{t}",
                          pipeline=False)

            # per-type work lists: each wave pairs a front tile and a back
            # tile so both ends of the group sequence finish early (the
            # bidirectional neighbor LSTM reads group t AND group tnb-1-t
            # at step t). Emission order must respect dataflow: a neighbor
            # step is emitted only once every column of both its groups has
            # been written by an emitted content chain.
            def build_seq(ty):
                m = TYPES[ty]
                seq = []
                for w in range(len(m["waves"])):
                    seq.append(("loads", w))
                    seq.append(("wave", w))
                return seq

            seqs = {ty: build_seq(ty) for ty in ("v", "a", "p")}
            xw_cur = {ty: None for ty in seqs}
            covered = {ty: np.zeros(TYPES[ty]["rows"], bool) for ty in seqs}
            nb_next = {ty: 0 for ty in seqs}

            def nb_ready(ty, t):
                mb = NB[ty]
                g0 = mb["goff"] + t * BC
                g1 = mb["goff"] + (mb["tnb"] - 1 - t) * BC
                cv = covered[ty]
                return cv[g0:g0 + BC].all() and cv[g1:g1 + BC].all()

            def emit_item(ty, item):
                m = TYPES[ty]
                if item[0] == "loads":
                    # one whole-row xbar-transpose DMA per wave: linear read
                    # of [wrows, re] -> [128, ts, wrows] (all timesteps)
                    w = item[1]
                    chunks = m["waves"][w]
                    st = stage[(ty, w)]
                    wrows = sum(N for _, N in chunks)
                    xt_t = xpool.tile([128, m["xslot"]], dt.bfloat16,
                                      name=f"xt_{ty}_{w}", tag=f"xt_{ty}",
                                      bufs=m["xbufs"])
                    v3 = xt_t[:, 0:m["ts"] * wrows].rearrange(
                        "p (t r) -> p t r", t=m["ts"])
                    nc.sync.dma_start(v3, st[0:wrows, :], transpose=True)
                    xw_cur[ty] = (w, v3)
                else:
                    # t-major over the whole wave: all chunks' step-t matmuls
                    # issue together, so the in-order PE queue never
                    # head-of-line-blocks on a single chain's recurrence
                    w = item[1]
                    w2, xw = xw_cur[ty]
                    assert w2 == w
                    chunks = m["waves"][w]
                    locs = []
                    loc = 0
                    for co, N in chunks:
                        locs.append(loc)
                        loc += N
                    hts = [hcpool.tile([128, 512], dt.bfloat16,
                                       name=f"h_{ty}_{co}", tag="h")
                           for co, N in chunks]
                    cts = [hcpool.tile([128, 512], dt.float32,
                                       name=f"c_{ty}_{co}", tag="c")
                           for co, N in chunks]
                    rts = m["ts"]
                    for t in range(rts):
                        for i, (co, N) in enumerate(chunks):
                            lstm_step(m["lc"], t,
                                      xw[:, t, locs[i]:locs[i] + N],
                                      xw[:, rts - 1 - t, locs[i]:locs[i] + N],
                                      hts[i], cts[i], hacc_segs(ty, co, N), N,
                                      f"c{ty}{t}_{co}")
                    for co, N in chunks:
                        covered[ty][co:co + N] = True
                    while (nb_next[ty] < NB[ty]["tnb"]
                           and nb_ready(ty, nb_next[ty])):
                        emit_nb_step(ty, nb_next[ty])
                        nb_next[ty] += 1

            # type-sequential emission (feed order); neighbor steps interleave
            # via emit_item's readiness checks
            for ty in ("v", "a", "p"):
                for item in seqs[ty]:
                    emit_item(ty, item)
            flush_tail()

            # ---------------- attention combine ----------------
            srcs = [hacc_a0[:, 0:BC], hacc_a0[:, 0:BC], hacc_nb["a"][:, 0:BC],
                    hacc_nb["p"][:, 0:BC], hacc_nb["v"][:, 0:BC]]
            ps_s = ppool.tile([128, 2048], dt.float32, name="ps_s", tag="ifog")
            for k in range(5):
                nc.tensor.matmul(ps_s[0:1, k * 256:k * 256 + BC], attw[:, k:k + 1],
                                 srcs[k], start=True, stop=True, skip_group_check=True)
            sb_s = const.tile([1, 5 * 256], dt.float32, name="sb_s", tag="sb_s")
            for k in range(5):
                nc.vector.tensor_copy(out=sb_s[0:1, k * 256:k * 256 + BC],
                                      in_=ps_s[0:1, k * 256:k * 256 + BC])
            lr = const.tile([1, 4 * 256], dt.float32, name="lr", tag="lr")
            nc.gpsimd.memset(lr[:], 0.0)
            for k in range(4):
                nc.vector.tensor_tensor(out=lr[0:1, k * 256:k * 256 + BC],
                                        in0=sb_s[0:1, 0:BC],
                                        in1=sb_s[0:1, (k + 1) * 256:(k + 1) * 256 + BC],
                                        op=OP.add)
            lr2 = const.tile([1, 4 * 256], dt.float32, name="lr2", tag="lr2")
            nc.vector.tensor_scalar_mul(lr2[:], lr[:], 0.01)
            nc.vector.tensor_tensor(out=lr2[:], in0=lr2[:], in1=lr[:], op=OP.max)
            ex = const.tile([1, 4 * 256], dt.float32, name="ex", tag="ex")
            nc.scalar.activation(ex[:], lr2[:], AF.Exp)
            zz = const.tile([1, 256], dt.float32, name="zz", tag="zz")
            nc.vector.tensor_tensor(out=zz[0:1, 0:BC], in0=ex[0:1, 0:BC],
                                    in1=ex[0:1, 256:256 + BC], op=OP.add)
            nc.vector.tensor_tensor(out=zz[0:1, 0:BC], in0=zz[0:1, 0:BC],
                                    in1=ex[0:1, 512:512 + BC], op=OP.add)
            nc.vector.tensor_tensor(out=zz[0:1, 0:BC], in0=zz[0:1, 0:BC],
                                    in1=ex[0:1, 768:768 + BC], op=OP.add)
            rz = const.tile([1, 256], dt.float32, name="rz", tag="rz")
            nc.vector.reciprocal(rz[0:1, 0:BC], zz[0:1, 0:BC])
            wk = const.tile([1, 4 * 256], dt.float32, name="wk", tag="wk")
            for k in range(4):
                nc.vector.tensor_tensor(out=wk[0:1, k * 256:k * 256 + BC],
                                        in0=ex[0:1, k * 256:k * 256 + BC],
                                        in1=rz[0:1, 0:BC], op=OP.mult)
            esrc = [hacc_a0[:, 0:BC], hacc_nb["a"][:, 0:BC],
                    hacc_nb["p"][:, 0:BC], hacc_nb["v"][:, 0:BC]]
            facc = const.tile([128, 256], dt.float32, name="facc", tag="facc")
            prod = const.tile([128, 256], dt.float32, name="prod", tag="prod")
            ps_w = ppool.tile([128, 2048], dt.float32, name="ps_w", tag="ifog")
            for k in range(4):
                nc.tensor.matmul(ps_w[:, k * 512:k * 512 + BC], wb_sc[k][0:1, :],
                                 wk[0:1, k * 256:k * 256 + BC], start=True, stop=True,
                                 skip_group_check=True)
            for k in range(4):
                nc.vector.tensor_tensor(out=prod[:, 0:BC],
                                        in0=ps_w[:, k * 512:k * 512 + BC],
                                        in1=esrc[k], op=OP.mult)
                if k == 0:
                    nc.vector.tensor_copy(out=facc[:, 0:BC], in_=prod[:, 0:BC])
                else:
                    nc.vector.tensor_tensor(out=facc[:, 0:BC], in0=facc[:, 0:BC],
                                            in1=prod[:, 0:BC], op=OP.add)
            ps_t = ppool.tile([128, 2048], dt.float32, name="ps_t", tag="ifog")
            ot0 = const.tile([128, 128], dt.float32, name="ot0", tag="ot0")
            nc.tensor.transpose(ps_t[:, 0:128], facc[:, 0:128], ident[:])
            nc.vector.tensor_copy(out=ot0[:], in_=ps_t[:, 0:128])
            nc.sync.dma_start(out_d[0:128, :], ot0[:])
            ot1 = const.tile([128, 128], dt.float32, name="ot1", tag="ot1")
            nc.tensor.transpose(ps_t[0:BC - 128, 512:640], facc[:, 128:BC], ident[:])
            nc.vector.tensor_copy(out=ot1[0:BC - 128, :], in_=ps_t[0:BC - 128, 512:640])
            nc.sync.dma_start(out_d[128:BC, :], ot1[0:BC - 128, :])

    nc.compile()
    return nc


# =========================== host side ===========================

def _pack_lstm(p, prescale=1.0):
    """torch-gate-order params -> (wih [128,512], whh [128,512], brow [1,512]).

    col layout per gate g in order [i, f, o, g(cell)]: fwd lhsT cols 0:64,
    bwd 64:128. whh is block-diag fwd/bwd, pre-transposed for lhsT use.
    """
    rows = {0: slice(0, 64), 1: slice(64, 128), 2: slice(192, 256), 3: slice(128, 192)}
    wih = np.zeros((128, 512), F32)
    whh = np.zeros((128, 512), F32)
    brow = np.zeros((1, 512), F32)
    for g in range(4):
        r = rows[g]
        wih[:, g * 128:g * 128 + 64] = np.asarray(p["Wih_f"], F32)[r].T * prescale
        wih[:, g * 128 + 64:(g + 1) * 128] = np.asarray(p["Wih_b"], F32)[r].T * prescale
        whh[0:64, g * 128:g * 128 + 64] = np.asarray(p["Whh_f"], F32)[r].T
        whh[64:128, g * 128 + 64:(g + 1) * 128] = np.asarray(p["Whh_b"], F32)[r].T
        brow[0, g * 128:g * 128 + 64] = np.asarray(p["b_f"], F32)[r]
        brow[0, g * 128 + 64:(g + 1) * 128] = np.asarray(p["b_b"], F32)[r]
    return wih, whh, brow


def _ids_to_tiles(ids, rows):
    full = np.zeros(rows, np.int64)
    full[:len(ids)] = np.asarray(ids, np.int64)
    return np.ascontiguousarray(full.reshape(rows // 128, 128).T.astype(np.int32))


def make_in_maps(feats, rnn_a_content, rnn_p_content, rnn_v_content,
                 rnn_a_neigh, rnn_p_neigh, rnn_v_neigh, a_neigh_att,
                 id_batch, a_neigh_ids, p_neigh_ids, v_neigh_ids):
    f = {k: np.asarray(v, F32) for k, v in feats.items()}
    a_tab = np.ascontiguousarray(np.concatenate(
        [f["a_coop"], f["a_net"], f["a_text"]], axis=1))
    p_tab = np.ascontiguousarray(np.concatenate(
        [f["p_t"], f["p_v_net"], f["p_a_net"], f["p_net"]], axis=1))
    v_tab = np.ascontiguousarray(np.concatenate(
        [f["v_net"], f["v_text"]], axis=1))

    packs = [
        _pack_lstm(rnn_a_content), _pack_lstm(rnn_p_content), _pack_lstm(rnn_v_content),
        _pack_lstm(rnn_a_neigh, 1.0 / TS_A), _pack_lstm(rnn_p_neigh, 1.0 / TS_P),
        _pack_lstm(rnn_v_neigh, 1.0 / TS_V),
    ]
    wih = np.concatenate([p[0] for p in packs], axis=1).astype(BF16)
    whh = np.concatenate([p[1] for p in packs], axis=1).astype(BF16)
    brow = np.concatenate([p[2] for p in packs], axis=1).astype(BF16)
    # per-partition bias vectors [128, 6*4]: col l*4+g = [b_fwd(64); b_bwd(64)]
    bvec = np.zeros((128, 24), F32)
    for l in range(6):
        br = packs[l][2][0]
        for g in range(4):
            bvec[0:64, l * 4 + g] = br[g * 128:g * 128 + 64]
            bvec[64:128, l * 4 + g] = br[g * 128 + 64:(g + 1) * 128]

    att = np.asarray(a_neigh_att, F32)          # [2d, 1]
    attA, attB = att[0:128, 0], att[128:256, 0]
    attw = np.stack([attA / TS_A, attB / TS_A, attB / A_NB, attB / P_NB,
                     attB / V_NB], axis=1).astype(F32)     # [128, 5]

    idb = np.asarray(id_batch, np.int64)
    anb = np.asarray(a_neigh_ids, np.int64)
    pnb = np.asarray(p_neigh_ids, np.int64)
    vnb = np.asarray(v_neigh_ids, np.int64)

    in_maps = []
    for c in range(NCORES):
        s = slice(c * BC, (c + 1) * BC)
        ids_a = np.concatenate([idb[s], anb[s].T.ravel()])   # slot-major (t-major)
        in_maps.append({
            "a_tab": a_tab, "p_tab": p_tab, "v_tab": v_tab,
            "idx_a": _ids_to_tiles(ids_a, ROWS_A),
            "idx_p": _ids_to_tiles(pnb[s].T.ravel(), ROWS_P),
            "idx_v": _ids_to_tiles(vnb[s].T.ravel(), ROWS_V),
            "wih_d": wih, "whh_d": whh, "brow_d": brow, "bvec_d": bvec.copy(),
            "att_d": attw,
        })
    return in_maps


def get_program():
    if "nc" not in _CACHE:
        _CACHE["nc"] = _build_program()
    return _CACHE["nc"]


def kernel(**inputs):
    from concourse import bass_utils

    nc = get_program()
    in_maps = make_in_maps(**inputs)
    res = bass_utils.run_bass_kernel_spmd(nc, in_maps, core_ids=list(range(NCORES)))
    out = np.concatenate([res.results[c]["out_d"] for c in range(NCORES)], axis=0)
    return np.asarray(out, F32)
